# revision 5
# baseline (speedup 1.0000x reference)
"""Bass/Trainium2 kernel for nn_GCF (2-layer GCN message passing + MLP).

Self-contained: takes FULL inputs, shards across 8 NeuronCores internally,
returns the FULL [16384] output.

Strategy:
  L1 (e1 = A @ ego): row-partitioned (18750 rows/core). Gathers of
    ego[col] via 4-queue SWDGE dma_gather (512B rows, int16-windowed);
    segment-sum via one-hot S matmuls accumulating PSUM row-tiles.
  L2 (only the ~32768 batch-needed rows of e2 = A @ e1): column-
    partitioned by e1-shard owner so every gather is core-local; computes
    transposed partials [100f x 32768pos] incl. e1 self-edges (vals
    pre-scaled by 1/3); one AllToAll (13MB) + local sum replaces any
    all-gather of e1.
  MLP: fused on-chip in transposed layout; ego[sel]/3 supplied by host
    (trivially data-parallel batch gather).
"""
import os
import sys

sys.path.insert(0, "/opt/trn_rl_repo")

import numpy as np

# -------------------- problem constants --------------------
P = 8               # cores
EMB = 100
DP = 128            # padded row width (512B)
GSZ = 1024          # max idxs per dma_gather
NQ = 4              # SWDGE queues
TB = 6              # L1 psum row-tiles per block

N_U = N_I = N_NODES = BATCH = R = WIN = NWIN = 0
T_L1 = NPOS = T_L2 = PERCORE = E1_ROWS = HALF_T = 0


def configure(n_u=100000, n_i=50000, batch=16384, win=30000):
    global N_U, N_I, N_NODES, BATCH, R, WIN, NWIN
    global T_L1, NPOS, T_L2, PERCORE, E1_ROWS, HALF_T
    N_U, N_I, BATCH, WIN = n_u, n_i, batch, win
    N_NODES = N_U + N_I
    R = N_NODES // P
    NWIN = -(-N_NODES // WIN)
    T_L1 = -(-R // 128)
    NPOS = 2 * BATCH
    T_L2 = NPOS // 128
    PERCORE = NPOS // P
    E1_ROWS = T_L1 * 128
    HALF_T = T_L2 // 2


configure()

_TRACE = bool(int(os.environ.get("GCF_TRACE", "0")))
LAST_EXEC_NS = [None]


# ======================================================================
# host-side schedule construction
# ======================================================================

def _group_slots(keys_per_core, vals_per_core, ngroups):
    """Pad per-(core,group) counts to a shared (max-over-cores, 128-mult)
    grid and place each core's edges into slot arrays."""
    counts = np.zeros((P, ngroups), np.int64)
    for m in range(P):
        counts[m] = np.bincount(keys_per_core[m], minlength=ngroups)
    ngrp = 128 * (-(-counts.max(axis=0) // 128))
    offs = np.concatenate([[0], np.cumsum(ngrp)])
    total = int(offs[-1])
    out = []
    for m in range(P):
        k = keys_per_core[m]
        order = np.argsort(k, kind="stable")
        ks = k[order]
        grp_start = np.concatenate(
            [[0], np.cumsum(np.bincount(ks, minlength=ngroups))])[:-1]
        rank = np.arange(len(ks)) - grp_start[ks]
        slots = offs[ks] + rank
        d = {}
        for name, arr in vals_per_core[m].items():
            full = np.zeros(total, arr.dtype)
            full[slots] = arr[order]
            d[name] = full
        mask = np.zeros(total, bool)
        mask[slots] = True
        d["_mask"] = mask
        out.append(d)
    return ngrp, offs, total, out


def _idx_layout_16(idx_flat):
    a = idx_flat.reshape(-1, 16).T
    return np.ascontiguousarray(np.tile(a, (8, 1)))


def _col_layout_128(flat):
    return np.ascontiguousarray(flat.reshape(-1, 128).T)


def build_host_data(user_emb, item_emb, adj_row, adj_col, adj_val,
                    userIdx, itemIdx, W1, b1, W2, b2, W3, b3):
    user_emb = np.asarray(user_emb, np.float32)
    item_emb = np.asarray(item_emb, np.float32)
    adj_row = np.asarray(adj_row, np.int64)
    adj_col = np.asarray(adj_col, np.int64)
    adj_val = np.asarray(adj_val, np.float32)
    userIdx = np.asarray(userIdx, np.int64)
    itemIdx = np.asarray(itemIdx, np.int64)

    ego = np.zeros((N_NODES, DP), np.float32)
    ego[:N_U, :EMB] = user_emb
    ego[N_U:, :EMB] = item_emb

    # ---------------- L1 schedule ----------------
    # group key, block-major: (block, w, tile-in-block) so that each
    # (block, w) run is slot-contiguous.
    NB1 = -(-T_L1 // TB)
    ngroups1 = NB1 * NWIN * TB

    def key1(t, w):
        return ((t // TB) * NWIN + w) * TB + (t % TB)

    core_of_edge = np.minimum(adj_row // R, P - 1)
    l1_keys, l1_vals = [], []
    for m in range(P):
        sel = np.nonzero(core_of_edge == m)[0]
        r_loc = adj_row[sel] - R * m
        t = r_loc // 128
        w = adj_col[sel] // WIN
        l1_keys.append(((t // TB) * NWIN + w) * TB + (t % TB))
        l1_vals.append({
            "idx": (adj_col[sel] - w * WIN).astype(np.int16),
            "rl": (r_loc % 128).astype(np.float32),
            "val": adj_val[sel],
        })
    ngrp1, offs1, T1, slots1 = _group_slots(l1_keys, l1_vals, ngroups1)
    for d in slots1:
        d["rl"][~d["_mask"]] = -1.0

    tile_nchunks = np.zeros(T_L1, np.int64)
    for t in range(T_L1):
        for w in range(NWIN):
            tile_nchunks[t] += int(ngrp1[key1(t, w)]) // 128

    # pieces: walk keys in order; break at window change or GSZ
    l1_pieces = []   # (window, slot_off, n_idx, [(tile, cloc, cglob)])
    cur, cur_off, cur_w = [], None, None
    for key in range(ngroups1):
        nch = int(ngrp1[key]) // 128
        if nch == 0:
            continue
        w = (key % (NWIN * TB)) // TB
        t = (key // (NWIN * TB)) * TB + key % TB
        for c in range(nch):
            if cur and (w != cur_w or len(cur) == GSZ // 128):
                l1_pieces.append((cur_w, cur_off, len(cur) * 128, cur))
                cur, cur_off = [], None
            if cur_off is None:
                cur_off, cur_w = int(offs1[key]) + c * 128, w
            cur.append((t, len(cur), int(offs1[key]) // 128 + c))
    if cur:
        l1_pieces.append((cur_w, cur_off, len(cur) * 128, cur))

    # ---------------- L2 schedule ----------------
    order = np.argsort(adj_row, kind="stable")
    csr_col = adj_col[order]
    csr_val = adj_val[order]
    deg = np.bincount(adj_row, minlength=N_NODES)
    csr_off = np.concatenate([[0], np.cumsum(deg)])

    rows_at_pos = np.empty(NPOS, np.int64)
    half_b = BATCH // P // 1
    for c in range(P):
        bs = slice(c * (BATCH // P), (c + 1) * (BATCH // P))
        rows_at_pos[c * PERCORE: c * PERCORE + PERCORE // 2] = userIdx[bs]
        rows_at_pos[c * PERCORE + PERCORE // 2: (c + 1) * PERCORE] = \
            N_U + itemIdx[bs]

    r = rows_at_pos
    cnt = deg[r]
    tot = int(cnt.sum())
    e_pos = np.repeat(np.arange(NPOS), cnt)
    within = np.arange(tot) - np.repeat(np.cumsum(cnt) - cnt, cnt)
    e_idx = np.repeat(csr_off[r], cnt) + within
    a_pos = np.concatenate([e_pos, np.arange(NPOS)])
    a_col = np.concatenate([csr_col[e_idx], r])
    a_val = np.concatenate([csr_val[e_idx] * (1.0 / 3.0),
                            np.full(NPOS, 1.0 / 3.0)]).astype(np.float32)

    owner = np.minimum(a_col // R, P - 1)
    loc = a_col - owner * R
    gidx = (loc % 128) * T_L1 + loc // 128
    l2_keys, l2_vals = [], []
    for k in range(P):
        sel = np.nonzero(owner == k)[0]
        l2_keys.append(a_pos[sel] // 128)
        l2_vals.append({
            "idx": gidx[sel].astype(np.int16),
            "rl": (a_pos[sel] % 128).astype(np.float32),
            "val": a_val[sel],
        })
    ngrp2, offs2, T2, slots2 = _group_slots(l2_keys, l2_vals, T_L2)
    for d in slots2:
        d["rl"][~d["_mask"]] = -1.0

    l2_pieces = []   # (half, slot_off, n_idx, [(tau, cloc, cglob)])
    for half in range(2):
        cur, cur_off = [], None
        for tau in range(half * HALF_T, (half + 1) * HALF_T):
            nch = int(ngrp2[tau]) // 128
            for c in range(nch):
                if cur and len(cur) == GSZ // 128:
                    l2_pieces.append((half, cur_off, len(cur) * 128, cur))
                    cur, cur_off = [], None
                if cur_off is None:
                    cur_off = int(offs2[tau]) + c * 128
                cur.append((tau, len(cur), int(offs2[tau]) // 128 + c))
        if cur:
            l2_pieces.append((half, cur_off, len(cur) * 128, cur))

    # ---------------- per-core input tensors ----------------
    iota = np.tile(np.arange(128, dtype=np.float32), (128, 1))
    w1 = np.asarray(W1, np.float32)
    per_core = []
    for m in range(P):
        d1, d2 = slots1[m], slots2[m]
        sel_rows = rows_at_pos[m * PERCORE:(m + 1) * PERCORE]
        ego_selT = np.zeros((128, PERCORE), np.float32)
        ego_selT[:EMB] = ego[sel_rows, :EMB].T * (1.0 / 3.0)
        per_core.append({
            "ego": ego,
            "l1_idx": _idx_layout_16(d1["idx"]),
            "l1_rl": _col_layout_128(d1["rl"]),
            "l1_val": _col_layout_128(d1["val"]),
            "l2_idx": _idx_layout_16(d2["idx"]),
            "l2_rl": _col_layout_128(d2["rl"]),
            "l2_val": _col_layout_128(d2["val"]),
            "ego_selT": ego_selT,
            "iota": iota,
            "w1u": np.ascontiguousarray(w1[:EMB]),
            "w1i": np.ascontiguousarray(w1[EMB:]),
            "w2": np.asarray(W2, np.float32),
            "w3": np.asarray(W3, np.float32),
            "b1": np.asarray(b1, np.float32).reshape(-1, 1),
            "b2": np.asarray(b2, np.float32).reshape(-1, 1),
            "b3": np.asarray(b3, np.float32).reshape(-1, 1),
        })

    sched = {
        "T1": T1, "T2": T2,
        "l1_pieces": l1_pieces, "l2_pieces": l2_pieces,
        "tile_nchunks": tile_nchunks, "ngrp2": ngrp2,
    }
    return sched, per_core


# ======================================================================
# bass program
# ======================================================================

def build_program(sched):
    from contextlib import ExitStack
    import concourse.bass as bass
    import concourse.tile as tile
    from concourse import bacc, mybir

    f32 = mybir.dt.float32
    i16 = mybir.dt.int16
    AF = mybir.ActivationFunctionType
    OP = mybir.AluOpType

    T1, T2 = sched["T1"], sched["T2"]
    l1_pieces, l2_pieces = sched["l1_pieces"], sched["l2_pieces"]
    tile_nchunks = sched["tile_nchunks"]
    ngrp2 = sched["ngrp2"]

    nc = bacc.Bacc("TRN2", target_bir_lowering=False, debug=False,
                   num_devices=P, num_swdge_queues=NQ)

    ego = nc.dram_tensor("ego", [N_NODES, DP], f32, kind="ExternalInput").ap()
    l1_idx = nc.dram_tensor("l1_idx", [128, T1 // 16], i16,
                            kind="ExternalInput").ap()
    l1_rl = nc.dram_tensor("l1_rl", [128, T1 // 128], f32,
                           kind="ExternalInput").ap()
    l1_val = nc.dram_tensor("l1_val", [128, T1 // 128], f32,
                            kind="ExternalInput").ap()
    l2_idx = nc.dram_tensor("l2_idx", [128, T2 // 16], i16,
                            kind="ExternalInput").ap()
    l2_rl = nc.dram_tensor("l2_rl", [128, T2 // 128], f32,
                           kind="ExternalInput").ap()
    l2_val = nc.dram_tensor("l2_val", [128, T2 // 128], f32,
                            kind="ExternalInput").ap()
    ego_selT = nc.dram_tensor("ego_selT", [128, PERCORE], f32,
                              kind="ExternalInput").ap()
    iota_in = nc.dram_tensor("iota", [128, 128], f32,
                             kind="ExternalInput").ap()
    w1u_in = nc.dram_tensor("w1u", [EMB, 64], f32, kind="ExternalInput").ap()
    w1i_in = nc.dram_tensor("w1i", [EMB, 64], f32, kind="ExternalInput").ap()
    w2_in = nc.dram_tensor("w2", [64, 32], f32, kind="ExternalInput").ap()
    w3_in = nc.dram_tensor("w3", [32, 1], f32, kind="ExternalInput").ap()
    b1_in = nc.dram_tensor("b1", [64, 1], f32, kind="ExternalInput").ap()
    b2_in = nc.dram_tensor("b2", [32, 1], f32, kind="ExternalInput").ap()
    b3_in = nc.dram_tensor("b3", [1, 1], f32, kind="ExternalInput").ap()
    out_d = nc.dram_tensor("out", [1, PERCORE // 2], f32,
                           kind="ExternalOutput").ap()

    qctr = [0]

    def next_q():
        q = qctr[0] % NQ
        qctr[0] += 1
        return q

    with tile.TileContext(nc) as tc, ExitStack() as top:
        const_p = top.enter_context(tc.tile_pool(name="const", bufs=1))
        iota_sb = const_p.tile([128, 128], f32, tag="iota")
        nc.sync.dma_start(iota_sb[:], iota_in[:])

        dram_p = top.enter_context(
            tc.tile_pool(name="dram", bufs=1, space="DRAM"))
        e1_hbm = dram_p.tile([E1_ROWS, DP], f32, tag="e1")
        a2a_in = dram_p.tile([P, EMB, PERCORE], f32, tag="a2ain")
        a2a_out = dram_p.tile([P, EMB, PERCORE], f32, tag="a2aout")

        # ---------------- L1 ----------------
        with ExitStack() as l1s:
            meta_p = l1s.enter_context(tc.tile_pool(name="l1meta", bufs=1))
            idx_sb = meta_p.tile([128, T1 // 16], i16, tag="idx1")
            nc.sync.dma_start(idx_sb[:], l1_idx[:])
            rl_sb = meta_p.tile([128, T1 // 128], f32, tag="rl1")
            nc.sync.dma_start(rl_sb[:], l1_rl[:])
            val_sb = meta_p.tile([128, T1 // 128], f32, tag="val1")
            nc.sync.dma_start(val_sb[:], l1_val[:])

            e1_p = l1s.enter_context(tc.tile_pool(name="e1sb", bufs=1))
            e1_sb = e1_p.tile([128, T_L1, EMB], f32, tag="e1sb")

            g_p = l1s.enter_context(tc.tile_pool(name="g1", bufs=8))
            s_p = l1s.enter_context(tc.tile_pool(name="s1", bufs=4))
            ps_p = l1s.enter_context(
                tc.tile_pool(name="ps1", bufs=8, space="PSUM"))

            for t in range(T_L1):
                if tile_nchunks[t] == 0:
                    nc.vector.memset(e1_sb[:, t, :], 0.0)

            psum_of = {}
            seen = {}
            for (w, off, n, chunks) in l1_pieces:
                g = g_p.tile([128, GSZ // 128, DP], f32, tag="g1")
                nc.gpsimd.dma_gather(
                    out_ap=g[:, :n // 128, :],
                    in_ap=ego[w * WIN:min((w + 1) * WIN, N_NODES), :],
                    idxs_ap=idx_sb[:, off // 16:(off + n) // 16],
                    num_idxs=n, num_idxs_reg=n, elem_size=DP,
                    queue_num=next_q(),
                )
                for (t, cloc, cglob) in chunks:
                    if t not in psum_of:
                        psum_of[t] = ps_p.tile([128, EMB], f32, tag="ps1", name="ps1t")
                        seen[t] = 0
                    s = s_p.tile([128, 128], f32, tag="s1")
                    nc.vector.tensor_scalar(
                        s[:], iota_sb[:],
                        rl_sb[:, cglob:cglob + 1],
                        val_sb[:, cglob:cglob + 1],
                        OP.is_equal, OP.mult)
                    seen[t] += 1
                    nc.tensor.matmul(
                        psum_of[t][:], s[:], g[:, cloc, :EMB],
                        start=(seen[t] == 1),
                        stop=(seen[t] == tile_nchunks[t]))
                    if seen[t] == tile_nchunks[t]:
                        nc.scalar.activation(e1_sb[:, t, :],
                                             psum_of[t][:], AF.Copy)
                        del psum_of[t]

            e1v = e1_hbm[:].rearrange("(p t) e -> p t e", p=128)
            nc.sync.dma_start(e1v[:, :, :EMB], e1_sb[:])

        # ---------------- L2 ----------------
        with ExitStack() as l2s:
            meta2 = l2s.enter_context(tc.tile_pool(name="l2meta", bufs=1))
            idx2_sb = meta2.tile([128, T2 // 16], i16, tag="idx2")
            nc.sync.dma_start(idx2_sb[:], l2_idx[:])
            rl2_sb = meta2.tile([128, T2 // 128], f32, tag="rl2")
            nc.sync.dma_start(rl2_sb[:], l2_rl[:])
            val2_sb = meta2.tile([128, T2 // 128], f32, tag="val2")
            nc.sync.dma_start(val2_sb[:], l2_val[:])

            g2_p = l2s.enter_context(tc.tile_pool(name="g2", bufs=8))
            s2_p = l2s.enter_context(tc.tile_pool(name="s2", bufs=4))
            ps2_p = l2s.enter_context(
                tc.tile_pool(name="ps2", bufs=4, space="PSUM"))
            part_p = l2s.enter_context(tc.tile_pool(name="part", bufs=2))

            for half in range(2):
                part = part_p.tile([128, HALF_T * 128], f32, tag="part")
                psum2 = {}
                seen2 = {}
                for (h, off, n, chunks) in l2_pieces:
                    if h != half:
                        continue
                    g = g2_p.tile([128, GSZ // 128, DP], f32, tag="g2")
                    nc.gpsimd.dma_gather(
                        out_ap=g[:, :n // 128, :],
                        in_ap=e1_hbm[:],
                        idxs_ap=idx2_sb[:, off // 16:(off + n) // 16],
                        num_idxs=n, num_idxs_reg=n, elem_size=DP,
                        queue_num=next_q(),
                    )
                    for (tau, cloc, cglob) in chunks:
                        tl = tau - half * HALF_T
                        if tau not in psum2:
                            psum2[tau] = ps2_p.tile([128, 128], f32,
                                                    tag="ps2", name="ps2t")
                            seen2[tau] = 0
                        s = s2_p.tile([128, 128], f32, tag="s2")
                        nc.vector.tensor_scalar(
                            s[:], iota_sb[:],
                            rl2_sb[:, cglob:cglob + 1],
                            val2_sb[:, cglob:cglob + 1],
                            OP.is_equal, OP.mult)
                        seen2[tau] += 1
                        nch = int(ngrp2[tau]) // 128
                        nc.tensor.matmul(
                            psum2[tau][:EMB, :], g[:, cloc, :EMB], s[:],
                            start=(seen2[tau] == 1),
                            stop=(seen2[tau] == nch))
                        if seen2[tau] == nch:
                            nc.scalar.activation(
                                part[:EMB, tl * 128:(tl + 1) * 128],
                                psum2[tau][:EMB, :], AF.Copy)
                            del psum2[tau]
                ndest = P // 2
                for dd in range(ndest):
                    nc.sync.dma_start(
                        a2a_in[ndest * half + dd],
                        part[:EMB, dd * PERCORE:(dd + 1) * PERCORE])

            nc.gpsimd.collective_compute(
                "AllToAll", mybir.AluOpType.bypass,
                replica_groups=[list(range(P))],
                ins=[a2a_in[:]],
                outs=[a2a_out[:]],
            )

        # ---------------- combine + MLP ----------------
        with ExitStack() as ms:
            acc_p = ms.enter_context(tc.tile_pool(name="acc", bufs=1))
            tmp_p = ms.enter_context(tc.tile_pool(name="tmp", bufs=2))
            mw_p = ms.enter_context(tc.tile_pool(name="mw", bufs=1))
            h_p = ms.enter_context(tc.tile_pool(name="h", bufs=1))
            ps1_p = ms.enter_context(
                tc.tile_pool(name="psm1", bufs=2, space="PSUM"))
            ps2m_p = ms.enter_context(
                tc.tile_pool(name="psm2", bufs=2, space="PSUM"))
            ps3_p = ms.enter_context(
                tc.tile_pool(name="psm3", bufs=2, space="PSUM"))

            acc = acc_p.tile([128, PERCORE], f32, tag="acc")
            nc.sync.dma_start(acc[:EMB, :], a2a_out[0])
            for i in range(1, P):
                tmp = tmp_p.tile([128, PERCORE], f32, tag="tmp")
                nc.sync.dma_start(tmp[:EMB, :], a2a_out[i])
                nc.vector.tensor_tensor(acc[:EMB, :], acc[:EMB, :],
                                        tmp[:EMB, :], op=OP.add)
            egot = tmp_p.tile([128, PERCORE], f32, tag="tmp")
            nc.sync.dma_start(egot[:], ego_selT[:])
            nc.vector.tensor_tensor(acc[:EMB, :], acc[:EMB, :],
                                    egot[:EMB, :], op=OP.add)

            w1u = mw_p.tile([EMB, 64], f32, tag="w1u")
            nc.sync.dma_start(w1u[:], w1u_in[:])
            w1i = mw_p.tile([EMB, 64], f32, tag="w1i")
            nc.sync.dma_start(w1i[:], w1i_in[:])
            w2 = mw_p.tile([64, 32], f32, tag="w2")
            nc.sync.dma_start(w2[:], w2_in[:])
            w3 = mw_p.tile([32, 1], f32, tag="w3")
            nc.sync.dma_start(w3[:], w3_in[:])
            b1 = mw_p.tile([64, 1], f32, tag="b1")
            nc.sync.dma_start(b1[:], b1_in[:])
            b2 = mw_p.tile([32, 1], f32, tag="b2")
            nc.sync.dma_start(b2[:], b2_in[:])
            b3 = mw_p.tile([1, 1], f32, tag="b3")
            nc.sync.dma_start(b3[:], b3_in[:])

            NB = PERCORE // 2
            MP = min(512, NB)
            h1 = h_p.tile([64, NB], f32, tag="h1")
            h2 = h_p.tile([32, NB], f32, tag="h2")
            h3 = h_p.tile([1, NB], f32, tag="h3")
            for npi in range(NB // MP):
                sl = slice(npi * MP, (npi + 1) * MP)
                ps1 = ps1_p.tile([64, MP], f32, tag="psm1")
                nc.tensor.matmul(ps1[:], w1u[:], acc[:EMB, sl],
                                 start=True, stop=False)
                nc.tensor.matmul(
                    ps1[:], w1i[:],
                    acc[:EMB, NB + npi * MP: NB + (npi + 1) * MP],
                    start=False, stop=True)
                nc.scalar.activation(h1[:, sl], ps1[:], AF.Relu,
                                     bias=b1[:])
                ps2 = ps2m_p.tile([32, MP], f32, tag="psm2")
                nc.tensor.matmul(ps2[:], w2[:], h1[:, sl],
                                 start=True, stop=True)
                nc.scalar.activation(h2[:, sl], ps2[:], AF.Identity,
                                     bias=b2[:])
                ps3 = ps3_p.tile([1, MP], f32, tag="psm3")
                nc.tensor.matmul(ps3[:], w3[:], h2[:, sl],
                                 start=True, stop=True)
                nc.scalar.activation(h3[:, sl], ps3[:], AF.Identity,
                                     bias=b3[:])
            nc.sync.dma_start(out_d[:], h3[:])

    nc.compile()
    return nc


# ======================================================================
# entry point
# ======================================================================

def kernel(**inputs):
    from concourse.bass_utils import run_bass_kernel_spmd

    sched, per_core = build_host_data(**inputs)
    nc = build_program(sched)

    if _TRACE:
        _install_ntff_hook()
    res = run_bass_kernel_spmd(nc, per_core, core_ids=list(range(P)),
                               trace=_TRACE)
    LAST_EXEC_NS[0] = res.exec_time_ns
    out = np.concatenate([res.results[m]["out"].reshape(-1)
                          for m in range(P)])
    return out.astype(np.float32)


def _install_ntff_hook():
    import types
    if "antenv.axon_hooks" not in sys.modules:
        mod = types.ModuleType("antenv.axon_hooks")
        _h = [None]
        mod.set_axon_ntff_profile_hook = lambda h: _h.__setitem__(0, h)
        mod.get_axon_ntff_profile_hook = lambda: _h[0]
        sys.modules["antenv.axon_hooks"] = mod
        import antenv
        antenv.axon_hooks = mod
    import antenv.axon_hooks as ah
    if ah.get_axon_ntff_profile_hook() is None:
        from trn_agent_boot.trn_boot import _ntff_profile_via_ctypes
        ah.set_axon_ntff_profile_hook(
            _ntff_profile_via_ctypes("/opt/axon/libaxon_pjrt.so"))


# revision 9
# speedup vs baseline: 4.8900x; 4.8900x over previous
"""Bass/Trainium2 kernel for nn_GCF (2-layer GCN message passing + MLP).

Self-contained: takes FULL inputs, shards across 8 NeuronCores internally,
returns the FULL [16384] output.

Strategy:
  L1 (e1 = A @ ego): row-partitioned (18750 rows/core). Gathers of
    ego[col] via 4-queue SWDGE dma_gather (512B rows, int16-windowed);
    segment-sum via one-hot S matmuls accumulating PSUM row-tiles.
  L2 (only the ~32768 batch-needed rows of e2 = A @ e1): column-
    partitioned by e1-shard owner so every gather is core-local; computes
    transposed partials [100f x 32768pos] incl. e1 self-edges (vals
    pre-scaled by 1/3); one AllToAll (13MB) + local sum replaces any
    all-gather of e1.
  MLP: fused on-chip in transposed layout; ego[sel]/3 supplied by host
    (trivially data-parallel batch gather).
"""
import os
import sys

sys.path.insert(0, "/opt/trn_rl_repo")

import numpy as np

# -------------------- problem constants --------------------
P = 8               # cores
EMB = 100
DP = 128            # padded row width (512B)
GSZ = 1024          # max idxs per dma_gather
NQ = 4              # SWDGE queues
TB = 6              # L1 psum row-tiles per block

N_U = N_I = N_NODES = BATCH = R = WIN = NWIN = RU = RI = 0
T_L1 = NPOS = T_L2 = PERCORE = E1_ROWS = HALF_T = 0


def configure(n_u=100000, n_i=50000, batch=16384, win=30000):
    global N_U, N_I, N_NODES, BATCH, R, WIN, NWIN, RU, RI
    global T_L1, NPOS, T_L2, PERCORE, E1_ROWS, HALF_T
    N_U, N_I, BATCH, WIN = n_u, n_i, batch, win
    N_NODES = N_U + N_I
    R = N_NODES // P
    RU = N_U // P
    RI = N_I // P
    NWIN = -(-N_NODES // WIN)
    T_L1 = -(-R // 128)
    NPOS = 2 * BATCH
    T_L2 = NPOS // 128
    PERCORE = NPOS // P
    E1_ROWS = T_L1 * 128
    HALF_T = T_L2 // 2


configure()

_TRACE = bool(int(os.environ.get("GCF_TRACE", "0")))
LAST_EXEC_NS = [None]


# ======================================================================
# host-side schedule construction
# ======================================================================

def _group_slots(keys_per_core, vals_per_core, ngroups):
    """Pad per-(core,group) counts to a shared (max-over-cores, 128-mult)
    grid and place each core's edges into slot arrays."""
    counts = np.zeros((P, ngroups), np.int64)
    for m in range(P):
        counts[m] = np.bincount(keys_per_core[m], minlength=ngroups)
    ngrp = 128 * (-(-counts.max(axis=0) // 128))
    offs = np.concatenate([[0], np.cumsum(ngrp)])
    total = int(offs[-1])
    out = []
    for m in range(P):
        k = keys_per_core[m]
        order = np.argsort(k, kind="stable")
        ks = k[order]
        grp_start = np.concatenate(
            [[0], np.cumsum(np.bincount(ks, minlength=ngroups))])[:-1]
        rank = np.arange(len(ks)) - grp_start[ks]
        slots = offs[ks] + rank
        d = {}
        for name, arr in vals_per_core[m].items():
            full = np.zeros(total, arr.dtype)
            full[slots] = arr[order]
            d[name] = full
        mask = np.zeros(total, bool)
        mask[slots] = True
        d["_mask"] = mask
        out.append(d)
    return ngrp, offs, total, out


def _node_core_loc(r):
    """Interleaved partition: core m owns users [m*RU,(m+1)*RU) at local
    rows [0,RU) and items [m*RI,(m+1)*RI) at local rows [RU,R). Keeps the
    column distribution identical across cores (padding stays small)."""
    is_item = r >= N_U
    core = np.where(is_item, (r - N_U) // RI, r // RU)
    loc = np.where(is_item, RU + (r - N_U) % RI, r % RU)
    return core, loc


def _idx_layout_16(idx_flat):
    a = idx_flat.reshape(-1, 16).T
    return np.ascontiguousarray(np.tile(a, (8, 1)))


def _col_layout_128(flat):
    return np.ascontiguousarray(flat.reshape(-1, 128).T)


def build_host_data(user_emb, item_emb, adj_row, adj_col, adj_val,
                    userIdx, itemIdx, W1, b1, W2, b2, W3, b3):
    user_emb = np.asarray(user_emb, np.float32)
    item_emb = np.asarray(item_emb, np.float32)
    adj_row = np.asarray(adj_row, np.int64)
    adj_col = np.asarray(adj_col, np.int64)
    adj_val = np.asarray(adj_val, np.float32)
    userIdx = np.asarray(userIdx, np.int64)
    itemIdx = np.asarray(itemIdx, np.int64)

    ego = np.zeros((N_NODES, DP), np.float32)
    ego[:N_U, :EMB] = user_emb
    ego[N_U:, :EMB] = item_emb

    # ---------------- L1 schedule ----------------
    # group key, block-major: (block, w, tile-in-block) so that each
    # (block, w) run is slot-contiguous.
    NB1 = -(-T_L1 // TB)
    ngroups1 = NB1 * NWIN * TB

    def key1(t, w):
        return ((t // TB) * NWIN + w) * TB + (t % TB)

    core_of_edge, r_loc_all = _node_core_loc(adj_row)
    l1_keys, l1_vals = [], []
    for m in range(P):
        sel = np.nonzero(core_of_edge == m)[0]
        r_loc = r_loc_all[sel]
        t = r_loc // 128
        w = adj_col[sel] // WIN
        l1_keys.append(((t // TB) * NWIN + w) * TB + (t % TB))
        l1_vals.append({
            "idx": (adj_col[sel] - w * WIN).astype(np.int16),
            "rl": (r_loc % 128).astype(np.float32),
            "val": adj_val[sel],
        })
    ngrp1, offs1, T1, slots1 = _group_slots(l1_keys, l1_vals, ngroups1)
    for d in slots1:
        d["rl"][~d["_mask"]] = -1.0

    tile_nchunks = np.zeros(T_L1, np.int64)
    for t in range(T_L1):
        for w in range(NWIN):
            tile_nchunks[t] += int(ngrp1[key1(t, w)]) // 128

    # pieces: walk keys in order; break at window change or GSZ
    l1_pieces = []   # (window, slot_off, n_idx, [(tile, cloc, cglob)])
    cur, cur_off, cur_w = [], None, None
    for key in range(ngroups1):
        nch = int(ngrp1[key]) // 128
        if nch == 0:
            continue
        w = (key % (NWIN * TB)) // TB
        t = (key // (NWIN * TB)) * TB + key % TB
        for c in range(nch):
            if cur and (w != cur_w or len(cur) == GSZ // 128):
                l1_pieces.append((cur_w, cur_off, len(cur) * 128, cur))
                cur, cur_off = [], None
            if cur_off is None:
                cur_off, cur_w = int(offs1[key]) + c * 128, w
            cur.append((t, len(cur), int(offs1[key]) // 128 + c))
    if cur:
        l1_pieces.append((cur_w, cur_off, len(cur) * 128, cur))

    # ---------------- L2 schedule ----------------
    order = np.argsort(adj_row, kind="stable")
    csr_col = adj_col[order]
    csr_val = adj_val[order]
    deg = np.bincount(adj_row, minlength=N_NODES)
    csr_off = np.concatenate([[0], np.cumsum(deg)])

    rows_at_pos = np.empty(NPOS, np.int64)
    half_b = BATCH // P // 1
    for c in range(P):
        bs = slice(c * (BATCH // P), (c + 1) * (BATCH // P))
        rows_at_pos[c * PERCORE: c * PERCORE + PERCORE // 2] = userIdx[bs]
        rows_at_pos[c * PERCORE + PERCORE // 2: (c + 1) * PERCORE] = \
            N_U + itemIdx[bs]

    r = rows_at_pos
    cnt = deg[r]
    tot = int(cnt.sum())
    e_pos = np.repeat(np.arange(NPOS), cnt)
    within = np.arange(tot) - np.repeat(np.cumsum(cnt) - cnt, cnt)
    e_idx = np.repeat(csr_off[r], cnt) + within
    a_pos = np.concatenate([e_pos, np.arange(NPOS)])
    a_col = np.concatenate([csr_col[e_idx], r])
    a_val = np.concatenate([csr_val[e_idx] * (1.0 / 3.0),
                            np.full(NPOS, 1.0 / 3.0)]).astype(np.float32)

    owner, loc = _node_core_loc(a_col)
    gidx = (loc % 128) * T_L1 + loc // 128
    l2_keys, l2_vals = [], []
    for k in range(P):
        sel = np.nonzero(owner == k)[0]
        l2_keys.append(a_pos[sel] // 128)
        l2_vals.append({
            "idx": gidx[sel].astype(np.int16),
            "rl": (a_pos[sel] % 128).astype(np.float32),
            "val": a_val[sel],
        })
    ngrp2, offs2, T2, slots2 = _group_slots(l2_keys, l2_vals, T_L2)
    for d in slots2:
        d["rl"][~d["_mask"]] = -1.0

    l2_pieces = []   # (half, slot_off, n_idx, [(tau, cloc, cglob)])
    for half in range(2):
        cur, cur_off = [], None
        for tau in range(half * HALF_T, (half + 1) * HALF_T):
            nch = int(ngrp2[tau]) // 128
            for c in range(nch):
                if cur and len(cur) == GSZ // 128:
                    l2_pieces.append((half, cur_off, len(cur) * 128, cur))
                    cur, cur_off = [], None
                if cur_off is None:
                    cur_off = int(offs2[tau]) + c * 128
                cur.append((tau, len(cur), int(offs2[tau]) // 128 + c))
        if cur:
            l2_pieces.append((half, cur_off, len(cur) * 128, cur))

    # ---------------- per-core input tensors ----------------
    iota = np.tile(np.arange(128, dtype=np.float32), (128, 1))
    w1 = np.asarray(W1, np.float32)
    per_core = []
    for m in range(P):
        d1, d2 = slots1[m], slots2[m]
        sel_rows = rows_at_pos[m * PERCORE:(m + 1) * PERCORE]
        ego_selT = np.zeros((128, PERCORE), np.float32)
        ego_selT[:EMB] = ego[sel_rows, :EMB].T * (1.0 / 3.0)
        per_core.append({
            "ego": ego,
            "l1_idx": _idx_layout_16(d1["idx"]),
            "l1_rl": _col_layout_128(d1["rl"]),
            "l1_val": _col_layout_128(d1["val"]),
            "l2_idx": _idx_layout_16(d2["idx"]),
            "l2_rl": _col_layout_128(d2["rl"]),
            "l2_val": _col_layout_128(d2["val"]),
            "ego_selT": ego_selT,
            "iota": iota, "iota8": iota8,
            "w1u": np.ascontiguousarray(w1[:EMB]),
            "w1i": np.ascontiguousarray(w1[EMB:]),
            "w2": np.asarray(W2, np.float32),
            "w3": np.asarray(W3, np.float32),
            "b1": np.asarray(b1, np.float32).reshape(-1, 1),
            "b2": np.asarray(b2, np.float32).reshape(-1, 1),
            "b3": np.asarray(b3, np.float32).reshape(-1, 1),
        })

    sched = {
        "T1": T1, "T2": T2,
        "l1_pieces": l1_pieces, "l2_pieces": l2_pieces,
        "tile_nchunks": tile_nchunks, "ngrp2": ngrp2,
    }
    return sched, per_core


# ======================================================================
# bass program
# ======================================================================

def build_program(sched):
    from contextlib import ExitStack
    import concourse.bass as bass
    import concourse.tile as tile
    from concourse import bacc, mybir

    f32 = mybir.dt.float32
    i16 = mybir.dt.int16
    AF = mybir.ActivationFunctionType
    OP = mybir.AluOpType

    T1, T2 = sched["T1"], sched["T2"]
    l1_pieces, l2_pieces = sched["l1_pieces"], sched["l2_pieces"]
    tile_nchunks = sched["tile_nchunks"]
    ngrp2 = sched["ngrp2"]

    nc = bacc.Bacc("TRN2", target_bir_lowering=False, debug=False,
                   num_devices=P, num_swdge_queues=NQ)

    ego = nc.dram_tensor("ego", [N_NODES, DP], f32, kind="ExternalInput").ap()
    l1_idx = nc.dram_tensor("l1_idx", [128, T1 // 16], i16,
                            kind="ExternalInput").ap()
    l1_rl = nc.dram_tensor("l1_rl", [128, T1 // 128], f32,
                           kind="ExternalInput").ap()
    l1_val = nc.dram_tensor("l1_val", [128, T1 // 128], f32,
                            kind="ExternalInput").ap()
    l2_idx = nc.dram_tensor("l2_idx", [128, T2 // 16], i16,
                            kind="ExternalInput").ap()
    l2_rl = nc.dram_tensor("l2_rl", [128, T2 // 128], f32,
                           kind="ExternalInput").ap()
    l2_val = nc.dram_tensor("l2_val", [128, T2 // 128], f32,
                            kind="ExternalInput").ap()
    ego_selT = nc.dram_tensor("ego_selT", [128, PERCORE], f32,
                              kind="ExternalInput").ap()
    iota_in = nc.dram_tensor("iota", [128, 128], f32,
                             kind="ExternalInput").ap()
    w1u_in = nc.dram_tensor("w1u", [EMB, 64], f32, kind="ExternalInput").ap()
    w1i_in = nc.dram_tensor("w1i", [EMB, 64], f32, kind="ExternalInput").ap()
    w2_in = nc.dram_tensor("w2", [64, 32], f32, kind="ExternalInput").ap()
    w3_in = nc.dram_tensor("w3", [32, 1], f32, kind="ExternalInput").ap()
    b1_in = nc.dram_tensor("b1", [64, 1], f32, kind="ExternalInput").ap()
    b2_in = nc.dram_tensor("b2", [32, 1], f32, kind="ExternalInput").ap()
    b3_in = nc.dram_tensor("b3", [1, 1], f32, kind="ExternalInput").ap()
    out_d = nc.dram_tensor("out", [1, PERCORE // 2], f32,
                           kind="ExternalOutput").ap()

    qctr = [0]

    def next_q():
        q = qctr[0] % NQ
        qctr[0] += 1
        return q

    with tile.TileContext(nc) as tc, ExitStack() as top:
        const_p = top.enter_context(tc.tile_pool(name="const", bufs=1))
        iota_sb = const_p.tile([128, 128], f32, tag="iota")
        nc.sync.dma_start(iota_sb[:], iota_in[:])

        dram_p = top.enter_context(
            tc.tile_pool(name="dram", bufs=1, space="DRAM"))
        e1_hbm = dram_p.tile([E1_ROWS, DP], f32, tag="e1")
        a2a_in = dram_p.tile([P, EMB, PERCORE], f32, tag="a2ain")
        a2a_out = dram_p.tile([P, EMB, PERCORE], f32, tag="a2aout")

        # ---------------- L1 ----------------
        with ExitStack() as l1s:
            meta_p = l1s.enter_context(tc.tile_pool(name="l1meta", bufs=1))
            idx_sb = meta_p.tile([128, T1 // 16], i16, tag="idx1")
            nc.sync.dma_start(idx_sb[:], l1_idx[:])
            rl_sb = meta_p.tile([128, T1 // 128], f32, tag="rl1")
            nc.sync.dma_start(rl_sb[:], l1_rl[:])
            val_sb = meta_p.tile([128, T1 // 128], f32, tag="val1")
            nc.sync.dma_start(val_sb[:], l1_val[:])

            e1_p = l1s.enter_context(tc.tile_pool(name="e1sb", bufs=1))
            e1_sb = e1_p.tile([128, T_L1, EMB], f32, tag="e1sb")

            g_p = l1s.enter_context(tc.tile_pool(name="g1", bufs=10))
            s_p = l1s.enter_context(tc.tile_pool(name="s1", bufs=4))
            ps_p = l1s.enter_context(
                tc.tile_pool(name="ps1", bufs=8, space="PSUM"))

            for t in range(T_L1):
                if tile_nchunks[t] == 0:
                    nc.vector.memset(e1_sb[:, t, :], 0.0)

            psum_of = {}
            seen = {}
            for (w, off, n, chunks) in l1_pieces:
                g = g_p.tile([128, GSZ // 128, DP], f32, tag="g1")
                nc.gpsimd.dma_gather(
                    out_ap=g[:, :n // 128, :],
                    in_ap=ego[w * WIN:min((w + 1) * WIN, N_NODES), :],
                    idxs_ap=idx_sb[:, off // 16:(off + n) // 16],
                    num_idxs=n, num_idxs_reg=n, elem_size=DP,
                    queue_num=next_q(),
                )
                for (t, cloc, cglob) in chunks:
                    if t not in psum_of:
                        psum_of[t] = ps_p.tile([128, EMB], f32, tag="ps1", name="ps1t")
                        seen[t] = 0
                    s = s_p.tile([128, 128], f32, tag="s1")
                    nc.vector.tensor_scalar(
                        s[:], iota_sb[:],
                        rl_sb[:, cglob:cglob + 1],
                        val_sb[:, cglob:cglob + 1],
                        OP.is_equal, OP.mult)
                    seen[t] += 1
                    nc.tensor.matmul(
                        psum_of[t][:], s[:], g[:, cloc, :EMB],
                        start=(seen[t] == 1),
                        stop=(seen[t] == tile_nchunks[t]))
                    if seen[t] == tile_nchunks[t]:
                        nc.scalar.activation(e1_sb[:, t, :],
                                             psum_of[t][:], AF.Copy)
                        del psum_of[t]

            e1v = e1_hbm[:].rearrange("(p t) e -> p t e", p=128)
            nc.sync.dma_start(e1v[:, :, :EMB], e1_sb[:])

        # ---------------- L2 ----------------
        with ExitStack() as l2s:
            meta2 = l2s.enter_context(tc.tile_pool(name="l2meta", bufs=1))
            idx2_sb = meta2.tile([128, T2 // 16], i16, tag="idx2")
            nc.sync.dma_start(idx2_sb[:], l2_idx[:])
            rl2_sb = meta2.tile([128, T2 // 128], f32, tag="rl2")
            nc.sync.dma_start(rl2_sb[:], l2_rl[:])
            val2_sb = meta2.tile([128, T2 // 128], f32, tag="val2")
            nc.sync.dma_start(val2_sb[:], l2_val[:])

            g2_p = l2s.enter_context(tc.tile_pool(name="g2", bufs=10))
            s2_p = l2s.enter_context(tc.tile_pool(name="s2", bufs=4))
            ps2_p = l2s.enter_context(
                tc.tile_pool(name="ps2", bufs=4, space="PSUM"))
            part_p = l2s.enter_context(tc.tile_pool(name="part", bufs=1))

            for half in range(2):
                part = part_p.tile([128, HALF_T * 128], f32, tag="part")
                psum2 = {}
                seen2 = {}
                for (h, off, n, chunks) in l2_pieces:
                    if h != half:
                        continue
                    g = g2_p.tile([128, GSZ // 128, DP], f32, tag="g2")
                    nc.gpsimd.dma_gather(
                        out_ap=g[:, :n // 128, :],
                        in_ap=e1_hbm[:],
                        idxs_ap=idx2_sb[:, off // 16:(off + n) // 16],
                        num_idxs=n, num_idxs_reg=n, elem_size=DP,
                        queue_num=next_q(),
                    )
                    for (tau, cloc, cglob) in chunks:
                        tl = tau - half * HALF_T
                        if tau not in psum2:
                            psum2[tau] = ps2_p.tile([128, 128], f32,
                                                    tag="ps2", name="ps2t")
                            seen2[tau] = 0
                        s = s2_p.tile([128, 128], f32, tag="s2")
                        nc.vector.tensor_scalar(
                            s[:], iota_sb[:],
                            rl2_sb[:, cglob:cglob + 1],
                            val2_sb[:, cglob:cglob + 1],
                            OP.is_equal, OP.mult)
                        seen2[tau] += 1
                        nch = int(ngrp2[tau]) // 128
                        nc.tensor.matmul(
                            psum2[tau][:EMB, :], g[:, cloc, :EMB], s[:],
                            start=(seen2[tau] == 1),
                            stop=(seen2[tau] == nch))
                        if seen2[tau] == nch:
                            nc.scalar.activation(
                                part[:EMB, tl * 128:(tl + 1) * 128],
                                psum2[tau][:EMB, :], AF.Copy)
                            del psum2[tau]
                ndest = P // 2
                for dd in range(ndest):
                    nc.sync.dma_start(
                        a2a_in[ndest * half + dd],
                        part[:EMB, dd * PERCORE:(dd + 1) * PERCORE])

            nc.gpsimd.collective_compute(
                "AllToAll", mybir.AluOpType.bypass,
                replica_groups=[list(range(P))],
                ins=[a2a_in[:]],
                outs=[a2a_out[:]],
            )

        # ---------------- combine + MLP ----------------
        with ExitStack() as ms:
            acc_p = ms.enter_context(tc.tile_pool(name="acc", bufs=1))
            tmp_p = ms.enter_context(tc.tile_pool(name="tmp", bufs=2))
            mw_p = ms.enter_context(tc.tile_pool(name="mw", bufs=1))
            h_p = ms.enter_context(tc.tile_pool(name="h", bufs=1))
            ps1_p = ms.enter_context(
                tc.tile_pool(name="psm1", bufs=2, space="PSUM"))
            ps2m_p = ms.enter_context(
                tc.tile_pool(name="psm2", bufs=2, space="PSUM"))
            ps3_p = ms.enter_context(
                tc.tile_pool(name="psm3", bufs=2, space="PSUM"))

            acc = acc_p.tile([128, PERCORE], f32, tag="acc")
            nc.sync.dma_start(acc[:EMB, :], a2a_out[0])
            for i in range(1, P):
                tmp = tmp_p.tile([128, PERCORE], f32, tag="tmp")
                nc.sync.dma_start(tmp[:EMB, :], a2a_out[i])
                nc.vector.tensor_tensor(acc[:EMB, :], acc[:EMB, :],
                                        tmp[:EMB, :], op=OP.add)
            egot = tmp_p.tile([128, PERCORE], f32, tag="tmp")
            nc.sync.dma_start(egot[:], ego_selT[:])
            nc.vector.tensor_tensor(acc[:EMB, :], acc[:EMB, :],
                                    egot[:EMB, :], op=OP.add)

            w1u = mw_p.tile([EMB, 64], f32, tag="w1u")
            nc.sync.dma_start(w1u[:], w1u_in[:])
            w1i = mw_p.tile([EMB, 64], f32, tag="w1i")
            nc.sync.dma_start(w1i[:], w1i_in[:])
            w2 = mw_p.tile([64, 32], f32, tag="w2")
            nc.sync.dma_start(w2[:], w2_in[:])
            w3 = mw_p.tile([32, 1], f32, tag="w3")
            nc.sync.dma_start(w3[:], w3_in[:])
            b1 = mw_p.tile([64, 1], f32, tag="b1")
            nc.sync.dma_start(b1[:], b1_in[:])
            b2 = mw_p.tile([32, 1], f32, tag="b2")
            nc.sync.dma_start(b2[:], b2_in[:])
            b3 = mw_p.tile([1, 1], f32, tag="b3")
            nc.sync.dma_start(b3[:], b3_in[:])

            NB = PERCORE // 2
            MP = min(512, NB)
            h1 = h_p.tile([64, NB], f32, tag="h1")
            h2 = h_p.tile([32, NB], f32, tag="h2")
            h3 = h_p.tile([1, NB], f32, tag="h3")
            for npi in range(NB // MP):
                sl = slice(npi * MP, (npi + 1) * MP)
                ps1 = ps1_p.tile([64, MP], f32, tag="psm1")
                nc.tensor.matmul(ps1[:], w1u[:], acc[:EMB, sl],
                                 start=True, stop=False)
                nc.tensor.matmul(
                    ps1[:], w1i[:],
                    acc[:EMB, NB + npi * MP: NB + (npi + 1) * MP],
                    start=False, stop=True)
                nc.scalar.activation(h1[:, sl], ps1[:], AF.Relu,
                                     bias=b1[:])
                ps2 = ps2m_p.tile([32, MP], f32, tag="psm2")
                nc.tensor.matmul(ps2[:], w2[:], h1[:, sl],
                                 start=True, stop=True)
                nc.scalar.activation(h2[:, sl], ps2[:], AF.Identity,
                                     bias=b2[:])
                ps3 = ps3_p.tile([1, MP], f32, tag="psm3")
                nc.tensor.matmul(ps3[:], w3[:], h2[:, sl],
                                 start=True, stop=True)
                nc.scalar.activation(h3[:, sl], ps3[:], AF.Identity,
                                     bias=b3[:])
            nc.sync.dma_start(out_d[:], h3[:])

    nc.compile()
    return nc


# ======================================================================
# entry point
# ======================================================================

def kernel(**inputs):
    from concourse.bass_utils import run_bass_kernel_spmd

    sched, per_core = build_host_data(**inputs)
    nc = build_program(sched)

    if _TRACE:
        _install_ntff_hook()
    res = run_bass_kernel_spmd(nc, per_core, core_ids=list(range(P)),
                               trace=_TRACE)
    LAST_EXEC_NS[0] = res.exec_time_ns
    out = np.concatenate([res.results[m]["out"].reshape(-1)
                          for m in range(P)])
    return out.astype(np.float32)


def _install_ntff_hook():
    import types
    if "antenv.axon_hooks" not in sys.modules:
        mod = types.ModuleType("antenv.axon_hooks")
        _h = [None]
        mod.set_axon_ntff_profile_hook = lambda h: _h.__setitem__(0, h)
        mod.get_axon_ntff_profile_hook = lambda: _h[0]
        sys.modules["antenv.axon_hooks"] = mod
        import antenv
        antenv.axon_hooks = mod
    import antenv.axon_hooks as ah
    if ah.get_axon_ntff_profile_hook() is None:
        from trn_agent_boot.trn_boot import _ntff_profile_via_ctypes
        ah.set_axon_ntff_profile_hook(
            _ntff_profile_via_ctypes("/opt/axon/libaxon_pjrt.so"))


# revision 10
# speedup vs baseline: 5.2843x; 1.0806x over previous
"""Bass/Trainium2 kernel for nn_GCF (2-layer GCN message passing + MLP).

Self-contained: takes FULL inputs, shards across 8 NeuronCores internally,
returns the FULL [16384] output.

Strategy:
  L1 (e1 = A @ ego): row-partitioned (18750 rows/core). Gathers of
    ego[col] via 4-queue SWDGE dma_gather (512B rows, int16-windowed);
    segment-sum via one-hot S matmuls accumulating PSUM row-tiles.
  L2 (only the ~32768 batch-needed rows of e2 = A @ e1): column-
    partitioned by e1-shard owner so every gather is core-local; computes
    transposed partials [100f x 32768pos] incl. e1 self-edges (vals
    pre-scaled by 1/3); one AllToAll (13MB) + local sum replaces any
    all-gather of e1.
  MLP: fused on-chip in transposed layout; ego[sel]/3 supplied by host
    (trivially data-parallel batch gather).
"""
import os
import sys

sys.path.insert(0, "/opt/trn_rl_repo")

import numpy as np

# -------------------- problem constants --------------------
P = 8               # cores
EMB = 100
DP = 128            # padded row width (512B)
GSZ = 1024          # max idxs per dma_gather
NQ = 4              # SWDGE queues
TB = 6              # L1 psum row-tiles per block

N_U = N_I = N_NODES = BATCH = R = WIN = NWIN = RU = RI = 0
T_L1 = NPOS = T_L2 = PERCORE = E1_ROWS = HALF_T = 0


def configure(n_u=100000, n_i=50000, batch=16384, win=30000):
    global N_U, N_I, N_NODES, BATCH, R, WIN, NWIN, RU, RI
    global T_L1, NPOS, T_L2, PERCORE, E1_ROWS, HALF_T
    N_U, N_I, BATCH, WIN = n_u, n_i, batch, win
    N_NODES = N_U + N_I
    R = N_NODES // P
    RU = N_U // P
    RI = N_I // P
    NWIN = -(-N_NODES // WIN)
    T_L1 = -(-R // 128)
    NPOS = 2 * BATCH
    T_L2 = NPOS // 128
    PERCORE = NPOS // P
    E1_ROWS = T_L1 * 128
    HALF_T = T_L2 // 2


configure()

_TRACE = bool(int(os.environ.get("GCF_TRACE", "0")))
LAST_EXEC_NS = [None]


# ======================================================================
# host-side schedule construction
# ======================================================================

def _group_slots(keys_per_core, vals_per_core, ngroups):
    """Pad per-(core,group) counts to a shared (max-over-cores, 128-mult)
    grid and place each core's edges into slot arrays."""
    counts = np.zeros((P, ngroups), np.int64)
    for m in range(P):
        counts[m] = np.bincount(keys_per_core[m], minlength=ngroups)
    ngrp = 128 * (-(-counts.max(axis=0) // 128))
    offs = np.concatenate([[0], np.cumsum(ngrp)])
    total = int(offs[-1])
    out = []
    for m in range(P):
        k = keys_per_core[m]
        order = np.argsort(k, kind="stable")
        ks = k[order]
        grp_start = np.concatenate(
            [[0], np.cumsum(np.bincount(ks, minlength=ngroups))])[:-1]
        rank = np.arange(len(ks)) - grp_start[ks]
        slots = offs[ks] + rank
        d = {}
        for name, arr in vals_per_core[m].items():
            full = np.zeros(total, arr.dtype)
            full[slots] = arr[order]
            d[name] = full
        mask = np.zeros(total, bool)
        mask[slots] = True
        d["_mask"] = mask
        out.append(d)
    return ngrp, offs, total, out


def _node_core_loc(r):
    """Interleaved partition: core m owns users [m*RU,(m+1)*RU) at local
    rows [0,RU) and items [m*RI,(m+1)*RI) at local rows [RU,R). Keeps the
    column distribution identical across cores (padding stays small)."""
    is_item = r >= N_U
    core = np.where(is_item, (r - N_U) // RI, r // RU)
    loc = np.where(is_item, RU + (r - N_U) % RI, r % RU)
    return core, loc


def _idx_layout_16(idx_flat):
    a = idx_flat.reshape(-1, 16).T
    return np.ascontiguousarray(np.tile(a, (8, 1)))


def _col_layout_128(flat):
    return np.ascontiguousarray(flat.reshape(-1, 128).T)


def build_host_data(user_emb, item_emb, adj_row, adj_col, adj_val,
                    userIdx, itemIdx, W1, b1, W2, b2, W3, b3):
    user_emb = np.asarray(user_emb, np.float32)
    item_emb = np.asarray(item_emb, np.float32)
    adj_row = np.asarray(adj_row, np.int64)
    adj_col = np.asarray(adj_col, np.int64)
    adj_val = np.asarray(adj_val, np.float32)
    userIdx = np.asarray(userIdx, np.int64)
    itemIdx = np.asarray(itemIdx, np.int64)

    ego = np.zeros((N_NODES, DP), np.float32)
    ego[:N_U, :EMB] = user_emb
    ego[N_U:, :EMB] = item_emb

    # ---------------- L1 schedule ----------------
    # group key, block-major: (block, w, tile-in-block) so that each
    # (block, w) run is slot-contiguous.
    NB1 = -(-T_L1 // TB)
    ngroups1 = NB1 * NWIN * TB

    def key1(t, w):
        return ((t // TB) * NWIN + w) * TB + (t % TB)

    core_of_edge, r_loc_all = _node_core_loc(adj_row)
    l1_keys, l1_vals = [], []
    for m in range(P):
        sel = np.nonzero(core_of_edge == m)[0]
        r_loc = r_loc_all[sel]
        t = r_loc // 128
        w = adj_col[sel] // WIN
        l1_keys.append(((t // TB) * NWIN + w) * TB + (t % TB))
        l1_vals.append({
            "idx": (adj_col[sel] - w * WIN).astype(np.int16),
            "rl": (r_loc % 128).astype(np.float32),
            "val": adj_val[sel],
        })
    ngrp1, offs1, T1, slots1 = _group_slots(l1_keys, l1_vals, ngroups1)
    for d in slots1:
        d["rl"][~d["_mask"]] = -1.0

    tile_nchunks = np.zeros(T_L1, np.int64)
    for t in range(T_L1):
        for w in range(NWIN):
            tile_nchunks[t] += int(ngrp1[key1(t, w)]) // 128

    # pieces: walk keys in order; break at window change or GSZ
    l1_pieces = []   # (window, slot_off, n_idx, [(tile, cloc, cglob)])
    cur, cur_off, cur_w = [], None, None
    for key in range(ngroups1):
        nch = int(ngrp1[key]) // 128
        if nch == 0:
            continue
        w = (key % (NWIN * TB)) // TB
        t = (key // (NWIN * TB)) * TB + key % TB
        for c in range(nch):
            if cur and (w != cur_w or len(cur) == GSZ // 128):
                l1_pieces.append((cur_w, cur_off, len(cur) * 128, cur))
                cur, cur_off = [], None
            if cur_off is None:
                cur_off, cur_w = int(offs1[key]) + c * 128, w
            cur.append((t, len(cur), int(offs1[key]) // 128 + c))
    if cur:
        l1_pieces.append((cur_w, cur_off, len(cur) * 128, cur))

    # ---------------- L2 schedule ----------------
    order = np.argsort(adj_row, kind="stable")
    csr_col = adj_col[order]
    csr_val = adj_val[order]
    deg = np.bincount(adj_row, minlength=N_NODES)
    csr_off = np.concatenate([[0], np.cumsum(deg)])

    rows_at_pos = np.empty(NPOS, np.int64)
    half_b = BATCH // P // 1
    for c in range(P):
        bs = slice(c * (BATCH // P), (c + 1) * (BATCH // P))
        rows_at_pos[c * PERCORE: c * PERCORE + PERCORE // 2] = userIdx[bs]
        rows_at_pos[c * PERCORE + PERCORE // 2: (c + 1) * PERCORE] = \
            N_U + itemIdx[bs]

    r = rows_at_pos
    cnt = deg[r]
    tot = int(cnt.sum())
    e_pos = np.repeat(np.arange(NPOS), cnt)
    within = np.arange(tot) - np.repeat(np.cumsum(cnt) - cnt, cnt)
    e_idx = np.repeat(csr_off[r], cnt) + within
    a_pos = np.concatenate([e_pos, np.arange(NPOS)])
    a_col = np.concatenate([csr_col[e_idx], r])
    a_val = np.concatenate([csr_val[e_idx] * (1.0 / 3.0),
                            np.full(NPOS, 1.0 / 3.0)]).astype(np.float32)

    owner, loc = _node_core_loc(a_col)
    gidx = (loc % 128) * T_L1 + loc // 128
    l2_keys, l2_vals = [], []
    for k in range(P):
        sel = np.nonzero(owner == k)[0]
        l2_keys.append(a_pos[sel] // 128)
        l2_vals.append({
            "idx": gidx[sel].astype(np.int16),
            "rl": (a_pos[sel] % 128).astype(np.float32),
            "val": a_val[sel],
        })
    ngrp2, offs2, T2, slots2 = _group_slots(l2_keys, l2_vals, T_L2)
    for d in slots2:
        d["rl"][~d["_mask"]] = -1.0

    l2_pieces = []   # (half, slot_off, n_idx, [(tau, cloc, cglob)])
    for half in range(2):
        cur, cur_off = [], None
        for tau in range(half * HALF_T, (half + 1) * HALF_T):
            nch = int(ngrp2[tau]) // 128
            for c in range(nch):
                if cur and len(cur) == GSZ // 128:
                    l2_pieces.append((half, cur_off, len(cur) * 128, cur))
                    cur, cur_off = [], None
                if cur_off is None:
                    cur_off = int(offs2[tau]) + c * 128
                cur.append((tau, len(cur), int(offs2[tau]) // 128 + c))
        if cur:
            l2_pieces.append((half, cur_off, len(cur) * 128, cur))

    # ---------------- per-core input tensors ----------------
    iota = np.tile(np.arange(128, dtype=np.float32), (128, 1))
    w1 = np.asarray(W1, np.float32)
    per_core = []
    for m in range(P):
        d1, d2 = slots1[m], slots2[m]
        sel_rows = rows_at_pos[m * PERCORE:(m + 1) * PERCORE]
        ego_selT = np.zeros((128, PERCORE), np.float32)
        ego_selT[:EMB] = ego[sel_rows, :EMB].T * (1.0 / 3.0)
        per_core.append({
            "ego": ego,
            "l1_idx": _idx_layout_16(d1["idx"]),
            "l1_rl": _col_layout_128(d1["rl"]),
            "l1_val": _col_layout_128(d1["val"]),
            "l2_idx": _idx_layout_16(d2["idx"]),
            "l2_rl": _col_layout_128(d2["rl"]),
            "l2_val": _col_layout_128(d2["val"]),
            "ego_selT": ego_selT,
            "iota": iota, "iota8": iota8,
            "w1u": np.ascontiguousarray(w1[:EMB]),
            "w1i": np.ascontiguousarray(w1[EMB:]),
            "w2": np.asarray(W2, np.float32),
            "w3": np.asarray(W3, np.float32),
            "b1": np.asarray(b1, np.float32).reshape(-1, 1),
            "b2": np.asarray(b2, np.float32).reshape(-1, 1),
            "b3": np.asarray(b3, np.float32).reshape(-1, 1),
        })

    sched = {
        "T1": T1, "T2": T2,
        "l1_pieces": l1_pieces, "l2_pieces": l2_pieces,
        "tile_nchunks": tile_nchunks, "ngrp2": ngrp2,
    }
    return sched, per_core


# ======================================================================
# bass program
# ======================================================================

def build_program(sched):
    from contextlib import ExitStack
    import concourse.bass as bass
    import concourse.tile as tile
    from concourse import bacc, mybir

    f32 = mybir.dt.float32
    i16 = mybir.dt.int16
    AF = mybir.ActivationFunctionType
    OP = mybir.AluOpType

    T1, T2 = sched["T1"], sched["T2"]
    l1_pieces, l2_pieces = sched["l1_pieces"], sched["l2_pieces"]
    tile_nchunks = sched["tile_nchunks"]
    ngrp2 = sched["ngrp2"]

    nc = bacc.Bacc("TRN2", target_bir_lowering=False, debug=False,
                   num_devices=P, num_swdge_queues=NQ)

    ego = nc.dram_tensor("ego", [N_NODES, DP], f32, kind="ExternalInput").ap()
    l1_idx = nc.dram_tensor("l1_idx", [128, T1 // 16], i16,
                            kind="ExternalInput").ap()
    l1_rl = nc.dram_tensor("l1_rl", [128, T1 // 128], f32,
                           kind="ExternalInput").ap()
    l1_val = nc.dram_tensor("l1_val", [128, T1 // 128], f32,
                            kind="ExternalInput").ap()
    l2_idx = nc.dram_tensor("l2_idx", [128, T2 // 16], i16,
                            kind="ExternalInput").ap()
    l2_rl = nc.dram_tensor("l2_rl", [128, T2 // 128], f32,
                           kind="ExternalInput").ap()
    l2_val = nc.dram_tensor("l2_val", [128, T2 // 128], f32,
                            kind="ExternalInput").ap()
    ego_selT = nc.dram_tensor("ego_selT", [128, PERCORE], f32,
                              kind="ExternalInput").ap()
    iota_in = nc.dram_tensor("iota", [128, 128], f32,
                             kind="ExternalInput").ap()
    w1u_in = nc.dram_tensor("w1u", [EMB, 64], f32, kind="ExternalInput").ap()
    w1i_in = nc.dram_tensor("w1i", [EMB, 64], f32, kind="ExternalInput").ap()
    w2_in = nc.dram_tensor("w2", [64, 32], f32, kind="ExternalInput").ap()
    w3_in = nc.dram_tensor("w3", [32, 1], f32, kind="ExternalInput").ap()
    b1_in = nc.dram_tensor("b1", [64, 1], f32, kind="ExternalInput").ap()
    b2_in = nc.dram_tensor("b2", [32, 1], f32, kind="ExternalInput").ap()
    b3_in = nc.dram_tensor("b3", [1, 1], f32, kind="ExternalInput").ap()
    out_d = nc.dram_tensor("out", [1, PERCORE // 2], f32,
                           kind="ExternalOutput").ap()

    qctr = [0]

    def next_q():
        q = qctr[0] % NQ
        qctr[0] += 1
        return q

    with tile.TileContext(nc) as tc, ExitStack() as top:
        const_p = top.enter_context(tc.tile_pool(name="const", bufs=1))
        iota_sb = const_p.tile([128, 128], f32, tag="iota")
        nc.sync.dma_start(iota_sb[:], iota_in[:])

        dram_p = top.enter_context(
            tc.tile_pool(name="dram", bufs=1, space="DRAM"))
        e1_hbm = dram_p.tile([E1_ROWS, DP], f32, tag="e1")
        a2a_in = dram_p.tile([P, EMB, PERCORE], bf, tag="a2ain")
        a2a_out = dram_p.tile([P, EMB, PERCORE], bf, tag="a2aout")

        # ---------------- L1 ----------------
        with ExitStack() as l1s:
            meta_p = l1s.enter_context(tc.tile_pool(name="l1meta", bufs=1))
            idx_sb = meta_p.tile([128, T1 // 16], i16, tag="idx1")
            nc.sync.dma_start(idx_sb[:], l1_idx[:])
            rl_sb = meta_p.tile([128, T1 // 128], f32, tag="rl1")
            nc.sync.dma_start(rl_sb[:], l1_rl[:])
            val_sb = meta_p.tile([128, T1 // 128], f32, tag="val1")
            nc.sync.dma_start(val_sb[:], l1_val[:])

            e1_p = l1s.enter_context(tc.tile_pool(name="e1sb", bufs=1))
            e1_sb = e1_p.tile([128, T_L1, EMB], f32, tag="e1sb")

            g_p = l1s.enter_context(tc.tile_pool(name="g1", bufs=10))
            s_p = l1s.enter_context(tc.tile_pool(name="s1", bufs=4))
            ps_p = l1s.enter_context(
                tc.tile_pool(name="ps1", bufs=8, space="PSUM"))

            for t in range(T_L1):
                if tile_nchunks[t] == 0:
                    nc.vector.memset(e1_sb[:, t, :], 0.0)

            psum_of = {}
            seen = {}
            for (w, off, n, chunks) in l1_pieces:
                g = g_p.tile([128, GSZ // 128, DP], f32, tag="g1")
                nc.gpsimd.dma_gather(
                    out_ap=g[:, :n // 128, :],
                    in_ap=ego[w * WIN:min((w + 1) * WIN, N_NODES), :],
                    idxs_ap=idx_sb[:, off // 16:(off + n) // 16],
                    num_idxs=n, num_idxs_reg=n, elem_size=DP,
                    queue_num=next_q(),
                )
                for (t, cloc, cglob) in chunks:
                    if t not in psum_of:
                        psum_of[t] = ps_p.tile([128, EMB], f32, tag="ps1", name="ps1t")
                        seen[t] = 0
                    s = s_p.tile([128, 128], f32, tag="s1")
                    nc.vector.tensor_scalar(
                        s[:], iota_sb[:],
                        rl_sb[:, cglob:cglob + 1],
                        val_sb[:, cglob:cglob + 1],
                        OP.is_equal, OP.mult)
                    seen[t] += 1
                    nc.tensor.matmul(
                        psum_of[t][:], s[:], g[:, cloc, :EMB],
                        start=(seen[t] == 1),
                        stop=(seen[t] == tile_nchunks[t]))
                    if seen[t] == tile_nchunks[t]:
                        nc.scalar.activation(e1_sb[:, t, :],
                                             psum_of[t][:], AF.Copy)
                        del psum_of[t]

            e1v = e1_hbm[:].rearrange("(p t) e -> p t e", p=128)
            nc.sync.dma_start(e1v[:, :, :EMB], e1_sb[:])

        # ---------------- L2 ----------------
        with ExitStack() as l2s:
            meta2 = l2s.enter_context(tc.tile_pool(name="l2meta", bufs=1))
            idx2_sb = meta2.tile([128, T2 // 16], i16, tag="idx2")
            nc.sync.dma_start(idx2_sb[:], l2_idx[:])
            rl2_sb = meta2.tile([128, T2 // 128], f32, tag="rl2")
            nc.sync.dma_start(rl2_sb[:], l2_rl[:])
            val2_sb = meta2.tile([128, T2 // 128], f32, tag="val2")
            nc.sync.dma_start(val2_sb[:], l2_val[:])

            g2_p = l2s.enter_context(tc.tile_pool(name="g2", bufs=10))
            s2_p = l2s.enter_context(tc.tile_pool(name="s2", bufs=4))
            ps2_p = l2s.enter_context(
                tc.tile_pool(name="ps2", bufs=4, space="PSUM"))
            part_p = l2s.enter_context(tc.tile_pool(name="part", bufs=1))

            for half in range(2):
                part = part_p.tile([128, HALF_T * 128], bf, tag="part")
                psum2 = {}
                seen2 = {}
                for (h, off, n, chunks) in l2_pieces:
                    if h != half:
                        continue
                    g = g2_p.tile([128, GSZ // 128, DP], f32, tag="g2")
                    nc.gpsimd.dma_gather(
                        out_ap=g[:, :n // 128, :],
                        in_ap=e1_hbm[:],
                        idxs_ap=idx2_sb[:, off // 16:(off + n) // 16],
                        num_idxs=n, num_idxs_reg=n, elem_size=DP,
                        queue_num=next_q(),
                    )
                    for (tau, cloc, cglob) in chunks:
                        tl = tau - half * HALF_T
                        if tau not in psum2:
                            psum2[tau] = ps2_p.tile([128, 128], f32,
                                                    tag="ps2", name="ps2t")
                            seen2[tau] = 0
                        s = s2_p.tile([128, 128], f32, tag="s2")
                        nc.vector.tensor_scalar(
                            s[:], iota_sb[:],
                            rl2_sb[:, cglob:cglob + 1],
                            val2_sb[:, cglob:cglob + 1],
                            OP.is_equal, OP.mult)
                        seen2[tau] += 1
                        nch = int(ngrp2[tau]) // 128
                        nc.tensor.matmul(
                            psum2[tau][:EMB, :], g[:, cloc, :EMB], s[:],
                            start=(seen2[tau] == 1),
                            stop=(seen2[tau] == nch))
                        if seen2[tau] == nch:
                            nc.scalar.activation(
                                part[:EMB, tl * 128:(tl + 1) * 128],
                                psum2[tau][:EMB, :], AF.Copy)
                            del psum2[tau]
                ndest = P // 2
                for dd in range(ndest):
                    nc.sync.dma_start(
                        a2a_in[ndest * half + dd],
                        part[:EMB, dd * PERCORE:(dd + 1) * PERCORE])

            nc.gpsimd.collective_compute(
                "AllToAll", mybir.AluOpType.bypass,
                replica_groups=[list(range(P))],
                ins=[a2a_in[:]],
                outs=[a2a_out[:]],
            )

        # ---------------- combine + MLP ----------------
        with ExitStack() as ms:
            acc_p = ms.enter_context(tc.tile_pool(name="acc", bufs=1))
            tmp_p = ms.enter_context(tc.tile_pool(name="tmp", bufs=2))
            mw_p = ms.enter_context(tc.tile_pool(name="mw", bufs=1))
            h_p = ms.enter_context(tc.tile_pool(name="h", bufs=1))
            ps1_p = ms.enter_context(
                tc.tile_pool(name="psm1", bufs=2, space="PSUM"))
            ps2m_p = ms.enter_context(
                tc.tile_pool(name="psm2", bufs=2, space="PSUM"))
            ps3_p = ms.enter_context(
                tc.tile_pool(name="psm3", bufs=2, space="PSUM"))

            acc = acc_p.tile([128, PERCORE], f32, tag="acc")
            egot = acc_p.tile([128, PERCORE], f32, tag="egot")
            nc.sync.dma_start(egot[:], ego_selT[:])
            tmp0 = tmp_p.tile([128, PERCORE], bf, tag="tmp")
            nc.sync.dma_start(tmp0[:EMB, :], a2a_out[0])
            nc.vector.tensor_tensor(acc[:EMB, :], egot[:EMB, :],
                                    tmp0[:EMB, :], op=OP.add)
            for i in range(1, P):
                tmp = tmp_p.tile([128, PERCORE], bf, tag="tmp")
                nc.sync.dma_start(tmp[:EMB, :], a2a_out[i])
                nc.vector.tensor_tensor(acc[:EMB, :], acc[:EMB, :],
                                        tmp[:EMB, :], op=OP.add)

            w1u = mw_p.tile([EMB, 64], f32, tag="w1u")
            nc.sync.dma_start(w1u[:], w1u_in[:])
            w1i = mw_p.tile([EMB, 64], f32, tag="w1i")
            nc.sync.dma_start(w1i[:], w1i_in[:])
            w2 = mw_p.tile([64, 32], f32, tag="w2")
            nc.sync.dma_start(w2[:], w2_in[:])
            w3 = mw_p.tile([32, 1], f32, tag="w3")
            nc.sync.dma_start(w3[:], w3_in[:])
            b1 = mw_p.tile([64, 1], f32, tag="b1")
            nc.sync.dma_start(b1[:], b1_in[:])
            b2 = mw_p.tile([32, 1], f32, tag="b2")
            nc.sync.dma_start(b2[:], b2_in[:])
            b3 = mw_p.tile([1, 1], f32, tag="b3")
            nc.sync.dma_start(b3[:], b3_in[:])

            NB = PERCORE // 2
            MP = min(512, NB)
            h1 = h_p.tile([64, NB], f32, tag="h1")
            h2 = h_p.tile([32, NB], f32, tag="h2")
            h3 = h_p.tile([1, NB], f32, tag="h3")
            for npi in range(NB // MP):
                sl = slice(npi * MP, (npi + 1) * MP)
                ps1 = ps1_p.tile([64, MP], f32, tag="psm1")
                nc.tensor.matmul(ps1[:], w1u[:], acc[:EMB, sl],
                                 start=True, stop=False)
                nc.tensor.matmul(
                    ps1[:], w1i[:],
                    acc[:EMB, NB + npi * MP: NB + (npi + 1) * MP],
                    start=False, stop=True)
                nc.scalar.activation(h1[:, sl], ps1[:], AF.Relu,
                                     bias=b1[:])
                ps2 = ps2m_p.tile([32, MP], f32, tag="psm2")
                nc.tensor.matmul(ps2[:], w2[:], h1[:, sl],
                                 start=True, stop=True)
                nc.scalar.activation(h2[:, sl], ps2[:], AF.Identity,
                                     bias=b2[:])
                ps3 = ps3_p.tile([1, MP], f32, tag="psm3")
                nc.tensor.matmul(ps3[:], w3[:], h2[:, sl],
                                 start=True, stop=True)
                nc.scalar.activation(h3[:, sl], ps3[:], AF.Identity,
                                     bias=b3[:])
            nc.sync.dma_start(out_d[:], h3[:])

    nc.compile()
    return nc


# ======================================================================
# entry point
# ======================================================================

def kernel(**inputs):
    from concourse.bass_utils import run_bass_kernel_spmd

    sched, per_core = build_host_data(**inputs)
    nc = build_program(sched)

    if _TRACE:
        _install_ntff_hook()
    res = run_bass_kernel_spmd(nc, per_core, core_ids=list(range(P)),
                               trace=_TRACE)
    LAST_EXEC_NS[0] = res.exec_time_ns
    out = np.concatenate([res.results[m]["out"].reshape(-1)
                          for m in range(P)])
    return out.astype(np.float32)


def _install_ntff_hook():
    import types
    if "antenv.axon_hooks" not in sys.modules:
        mod = types.ModuleType("antenv.axon_hooks")
        _h = [None]
        mod.set_axon_ntff_profile_hook = lambda h: _h.__setitem__(0, h)
        mod.get_axon_ntff_profile_hook = lambda: _h[0]
        sys.modules["antenv.axon_hooks"] = mod
        import antenv
        antenv.axon_hooks = mod
    import antenv.axon_hooks as ah
    if ah.get_axon_ntff_profile_hook() is None:
        from trn_agent_boot.trn_boot import _ntff_profile_via_ctypes
        ah.set_axon_ntff_profile_hook(
            _ntff_profile_via_ctypes("/opt/axon/libaxon_pjrt.so"))


# revision 11
# speedup vs baseline: 5.3936x; 1.0207x over previous
"""Bass/Trainium2 kernel for nn_GCF (2-layer GCN message passing + MLP).

Self-contained: takes FULL inputs, shards across 8 NeuronCores internally,
returns the FULL [16384] output.

Strategy:
  L1 (e1 = A @ ego): row-partitioned (18750 rows/core). Gathers of
    ego[col] via 4-queue SWDGE dma_gather (512B rows, int16-windowed);
    segment-sum via one-hot S matmuls accumulating PSUM row-tiles.
  L2 (only the ~32768 batch-needed rows of e2 = A @ e1): column-
    partitioned by e1-shard owner so every gather is core-local; computes
    transposed partials [100f x 32768pos] incl. e1 self-edges (vals
    pre-scaled by 1/3); one AllToAll (13MB) + local sum replaces any
    all-gather of e1.
  MLP: fused on-chip in transposed layout; ego[sel]/3 supplied by host
    (trivially data-parallel batch gather).
"""
import os
import sys

sys.path.insert(0, "/opt/trn_rl_repo")

import numpy as np

# -------------------- problem constants --------------------
P = 8               # cores
EMB = 100
DP = 128            # padded row width (512B)
GSZ = 1024          # max idxs per dma_gather
NQ = 4              # SWDGE queues
TB = 6              # L1 psum row-tiles per block

N_U = N_I = N_NODES = BATCH = R = WIN = NWIN = RU = RI = 0
T_L1 = NPOS = T_L2 = PERCORE = E1_ROWS = HALF_T = 0


def configure(n_u=100000, n_i=50000, batch=16384, win=30000):
    global N_U, N_I, N_NODES, BATCH, R, WIN, NWIN, RU, RI
    global T_L1, NPOS, T_L2, PERCORE, E1_ROWS, HALF_T
    N_U, N_I, BATCH, WIN = n_u, n_i, batch, win
    N_NODES = N_U + N_I
    R = N_NODES // P
    RU = N_U // P
    RI = N_I // P
    NWIN = -(-N_NODES // WIN)
    T_L1 = -(-R // 128)
    NPOS = 2 * BATCH
    T_L2 = NPOS // 128
    PERCORE = NPOS // P
    E1_ROWS = T_L1 * 128
    HALF_T = T_L2 // 2


configure()

_TRACE = bool(int(os.environ.get("GCF_TRACE", "0")))
LAST_EXEC_NS = [None]


# ======================================================================
# host-side schedule construction
# ======================================================================

def _group_slots(keys_per_core, vals_per_core, ngroups):
    """Pad per-(core,group) counts to a shared (max-over-cores, 128-mult)
    grid and place each core's edges into slot arrays."""
    counts = np.zeros((P, ngroups), np.int64)
    for m in range(P):
        counts[m] = np.bincount(keys_per_core[m], minlength=ngroups)
    ngrp = 128 * (-(-counts.max(axis=0) // 128))
    offs = np.concatenate([[0], np.cumsum(ngrp)])
    total = int(offs[-1])
    out = []
    for m in range(P):
        k = keys_per_core[m]
        order = np.argsort(k, kind="stable")
        ks = k[order]
        grp_start = np.concatenate(
            [[0], np.cumsum(np.bincount(ks, minlength=ngroups))])[:-1]
        rank = np.arange(len(ks)) - grp_start[ks]
        slots = offs[ks] + rank
        d = {}
        for name, arr in vals_per_core[m].items():
            full = np.zeros(total, arr.dtype)
            full[slots] = arr[order]
            d[name] = full
        mask = np.zeros(total, bool)
        mask[slots] = True
        d["_mask"] = mask
        out.append(d)
    return ngrp, offs, total, out


def _node_core_loc(r):
    """Interleaved partition: core m owns users [m*RU,(m+1)*RU) at local
    rows [0,RU) and items [m*RI,(m+1)*RI) at local rows [RU,R). Keeps the
    column distribution identical across cores (padding stays small)."""
    is_item = r >= N_U
    core = np.where(is_item, (r - N_U) // RI, r // RU)
    loc = np.where(is_item, RU + (r - N_U) % RI, r % RU)
    return core, loc


def _idx_layout_16(idx_flat):
    a = idx_flat.reshape(-1, 16).T
    return np.ascontiguousarray(np.tile(a, (8, 1)))


def _col_layout_128(flat):
    return np.ascontiguousarray(flat.reshape(-1, 128).T)


def build_host_data(user_emb, item_emb, adj_row, adj_col, adj_val,
                    userIdx, itemIdx, W1, b1, W2, b2, W3, b3):
    user_emb = np.asarray(user_emb, np.float32)
    item_emb = np.asarray(item_emb, np.float32)
    adj_row = np.asarray(adj_row, np.int64)
    adj_col = np.asarray(adj_col, np.int64)
    adj_val = np.asarray(adj_val, np.float32)
    userIdx = np.asarray(userIdx, np.int64)
    itemIdx = np.asarray(itemIdx, np.int64)

    ego = np.zeros((N_NODES, DP), np.float32)
    ego[:N_U, :EMB] = user_emb
    ego[N_U:, :EMB] = item_emb

    # ---------------- L1 schedule ----------------
    # group key, block-major: (block, w, tile-in-block) so that each
    # (block, w) run is slot-contiguous.
    NB1 = -(-T_L1 // TB)
    ngroups1 = NB1 * NWIN * TB

    def key1(t, w):
        return ((t // TB) * NWIN + w) * TB + (t % TB)

    core_of_edge, r_loc_all = _node_core_loc(adj_row)
    l1_keys, l1_vals = [], []
    for m in range(P):
        sel = np.nonzero(core_of_edge == m)[0]
        r_loc = r_loc_all[sel]
        t = r_loc // 128
        w = adj_col[sel] // WIN
        l1_keys.append(((t // TB) * NWIN + w) * TB + (t % TB))
        l1_vals.append({
            "idx": (adj_col[sel] - w * WIN).astype(np.int16),
            "rl": (r_loc % 128).astype(np.float32),
            "val": adj_val[sel],
        })
    ngrp1, offs1, T1, slots1 = _group_slots(l1_keys, l1_vals, ngroups1)
    for d in slots1:
        d["rl"][~d["_mask"]] = -1.0

    tile_nchunks = np.zeros(T_L1, np.int64)
    for t in range(T_L1):
        for w in range(NWIN):
            tile_nchunks[t] += int(ngrp1[key1(t, w)]) // 128

    # pieces: walk keys in order; break at window change or GSZ
    l1_pieces = []   # (window, slot_off, n_idx, [(tile, cloc, cglob)])
    cur, cur_off, cur_w = [], None, None
    for key in range(ngroups1):
        nch = int(ngrp1[key]) // 128
        if nch == 0:
            continue
        w = (key % (NWIN * TB)) // TB
        t = (key // (NWIN * TB)) * TB + key % TB
        for c in range(nch):
            if cur and (w != cur_w or len(cur) == GSZ // 128):
                l1_pieces.append((cur_w, cur_off, len(cur) * 128, cur))
                cur, cur_off = [], None
            if cur_off is None:
                cur_off, cur_w = int(offs1[key]) + c * 128, w
            cur.append((t, len(cur), int(offs1[key]) // 128 + c))
    if cur:
        l1_pieces.append((cur_w, cur_off, len(cur) * 128, cur))

    # ---------------- L2 schedule ----------------
    order = np.argsort(adj_row, kind="stable")
    csr_col = adj_col[order]
    csr_val = adj_val[order]
    deg = np.bincount(adj_row, minlength=N_NODES)
    csr_off = np.concatenate([[0], np.cumsum(deg)])

    rows_at_pos = np.empty(NPOS, np.int64)
    half_b = BATCH // P // 1
    for c in range(P):
        bs = slice(c * (BATCH // P), (c + 1) * (BATCH // P))
        rows_at_pos[c * PERCORE: c * PERCORE + PERCORE // 2] = userIdx[bs]
        rows_at_pos[c * PERCORE + PERCORE // 2: (c + 1) * PERCORE] = \
            N_U + itemIdx[bs]

    r = rows_at_pos
    cnt = deg[r]
    tot = int(cnt.sum())
    e_pos = np.repeat(np.arange(NPOS), cnt)
    within = np.arange(tot) - np.repeat(np.cumsum(cnt) - cnt, cnt)
    e_idx = np.repeat(csr_off[r], cnt) + within
    a_pos = np.concatenate([e_pos, np.arange(NPOS)])
    a_col = np.concatenate([csr_col[e_idx], r])
    a_val = np.concatenate([csr_val[e_idx] * (1.0 / 3.0),
                            np.full(NPOS, 1.0 / 3.0)]).astype(np.float32)

    owner, loc = _node_core_loc(a_col)
    gidx = (loc % 128) * T_L1 + loc // 128
    l2_keys, l2_vals = [], []
    for k in range(P):
        sel = np.nonzero(owner == k)[0]
        l2_keys.append(a_pos[sel] // 128)
        l2_vals.append({
            "idx": gidx[sel].astype(np.int16),
            "rl": (a_pos[sel] % 128).astype(np.float32),
            "val": a_val[sel],
        })
    ngrp2, offs2, T2, slots2 = _group_slots(l2_keys, l2_vals, T_L2)
    for d in slots2:
        d["rl"][~d["_mask"]] = -1.0

    l2_pieces = []   # (half, slot_off, n_idx, [(tau, cloc, cglob)])
    for half in range(2):
        cur, cur_off = [], None
        for tau in range(half * HALF_T, (half + 1) * HALF_T):
            nch = int(ngrp2[tau]) // 128
            for c in range(nch):
                if cur and len(cur) == GSZ // 128:
                    l2_pieces.append((half, cur_off, len(cur) * 128, cur))
                    cur, cur_off = [], None
                if cur_off is None:
                    cur_off = int(offs2[tau]) + c * 128
                cur.append((tau, len(cur), int(offs2[tau]) // 128 + c))
        if cur:
            l2_pieces.append((half, cur_off, len(cur) * 128, cur))

    # ---------------- per-core input tensors ----------------
    iota = np.tile(np.arange(128, dtype=np.float32), (128, 1))
    w1 = np.asarray(W1, np.float32)
    per_core = []
    for m in range(P):
        d1, d2 = slots1[m], slots2[m]
        sel_rows = rows_at_pos[m * PERCORE:(m + 1) * PERCORE]
        ego_selT = np.zeros((128, PERCORE), np.float32)
        ego_selT[:EMB] = ego[sel_rows, :EMB].T * (1.0 / 3.0)
        per_core.append({
            "ego": ego,
            "l1_idx": _idx_layout_16(d1["idx"]),
            "l1_rl": _col_layout_128(d1["rl"]),
            "l1_val": _col_layout_128(d1["val"]),
            "l2_idx": _idx_layout_16(d2["idx"]),
            "l2_rl": _col_layout_128(d2["rl"]),
            "l2_val": _col_layout_128(d2["val"]),
            "ego_selT": ego_selT,
            "iota": iota, "iota8": iota8,
            "w1u": np.ascontiguousarray(w1[:EMB]),
            "w1i": np.ascontiguousarray(w1[EMB:]),
            "w2": np.asarray(W2, np.float32),
            "w3": np.asarray(W3, np.float32),
            "b1": np.asarray(b1, np.float32).reshape(-1, 1),
            "b2": np.asarray(b2, np.float32).reshape(-1, 1),
            "b3": np.asarray(b3, np.float32).reshape(-1, 1),
        })

    sched = {
        "T1": T1, "T2": T2,
        "l1_pieces": l1_pieces, "l2_pieces": l2_pieces,
        "tile_nchunks": tile_nchunks, "ngrp2": ngrp2,
    }
    return sched, per_core


# ======================================================================
# bass program
# ======================================================================

def build_program(sched):
    from contextlib import ExitStack
    import concourse.bass as bass
    import concourse.tile as tile
    from concourse import bacc, mybir

    f32 = mybir.dt.float32
    i16 = mybir.dt.int16
    AF = mybir.ActivationFunctionType
    OP = mybir.AluOpType

    T1, T2 = sched["T1"], sched["T2"]
    l1_pieces, l2_pieces = sched["l1_pieces"], sched["l2_pieces"]
    tile_nchunks = sched["tile_nchunks"]
    ngrp2 = sched["ngrp2"]

    nc = bacc.Bacc("TRN2", target_bir_lowering=False, debug=False,
                   num_devices=P, num_swdge_queues=NQ)

    ego = nc.dram_tensor("ego", [N_NODES, DP], f32, kind="ExternalInput").ap()
    l1_idx = nc.dram_tensor("l1_idx", [128, T1 // 16], i16,
                            kind="ExternalInput").ap()
    l1_rl = nc.dram_tensor("l1_rl", [128, T1 // 128], f32,
                           kind="ExternalInput").ap()
    l1_val = nc.dram_tensor("l1_val", [128, T1 // 128], f32,
                            kind="ExternalInput").ap()
    l2_idx = nc.dram_tensor("l2_idx", [128, T2 // 16], i16,
                            kind="ExternalInput").ap()
    l2_rl = nc.dram_tensor("l2_rl", [128, T2 // 128], f32,
                           kind="ExternalInput").ap()
    l2_val = nc.dram_tensor("l2_val", [128, T2 // 128], f32,
                            kind="ExternalInput").ap()
    ego_selT = nc.dram_tensor("ego_selT", [128, PERCORE], f32,
                              kind="ExternalInput").ap()
    iota_in = nc.dram_tensor("iota", [128, 128], f32,
                             kind="ExternalInput").ap()
    w1u_in = nc.dram_tensor("w1u", [EMB, 64], f32, kind="ExternalInput").ap()
    w1i_in = nc.dram_tensor("w1i", [EMB, 64], f32, kind="ExternalInput").ap()
    w2_in = nc.dram_tensor("w2", [64, 32], f32, kind="ExternalInput").ap()
    w3_in = nc.dram_tensor("w3", [32, 1], f32, kind="ExternalInput").ap()
    b1_in = nc.dram_tensor("b1", [64, 1], f32, kind="ExternalInput").ap()
    b2_in = nc.dram_tensor("b2", [32, 1], f32, kind="ExternalInput").ap()
    b3_in = nc.dram_tensor("b3", [1, 1], f32, kind="ExternalInput").ap()
    out_d = nc.dram_tensor("out", [1, PERCORE // 2], f32,
                           kind="ExternalOutput").ap()

    qctr = [0]

    def next_q():
        q = qctr[0] % NQ
        qctr[0] += 1
        return q

    with tile.TileContext(nc) as tc, ExitStack() as top:
        const_p = top.enter_context(tc.tile_pool(name="const", bufs=1))
        iota_sb = const_p.tile([128, 128], f32, tag="iota")
        nc.sync.dma_start(iota_sb[:], iota_in[:])

        dram_p = top.enter_context(
            tc.tile_pool(name="dram", bufs=1, space="DRAM"))
        e1_hbm = dram_p.tile([E1_ROWS, DP], f32, tag="e1")
        a2a_in = dram_p.tile([P, EMB, PERCORE], bf, tag="a2ain")
        a2a_out = dram_p.tile([P, EMB, PERCORE], bf, tag="a2aout")

        # ---------------- L1 ----------------
        with ExitStack() as l1s:
            meta_p = l1s.enter_context(tc.tile_pool(name="l1meta", bufs=1))
            idx_sb = meta_p.tile([128, T1 // 16], i16, tag="idx1")
            nc.sync.dma_start(idx_sb[:], l1_idx[:])
            rl_sb = meta_p.tile([128, T1 // 128], f32, tag="rl1")
            nc.sync.dma_start(rl_sb[:], l1_rl[:])
            val_sb = meta_p.tile([128, T1 // 128], f32, tag="val1")
            nc.sync.dma_start(val_sb[:], l1_val[:])

            e1_p = l1s.enter_context(tc.tile_pool(name="e1sb", bufs=1))
            e1_sb = e1_p.tile([128, T_L1, EMB], f32, tag="e1sb")

            g_p = l1s.enter_context(tc.tile_pool(name="g1", bufs=16))
            s_p = l1s.enter_context(tc.tile_pool(name="s1", bufs=4))
            ps_p = l1s.enter_context(
                tc.tile_pool(name="ps1", bufs=8, space="PSUM"))

            for t in range(T_L1):
                if tile_nchunks[t] == 0:
                    nc.vector.memset(e1_sb[:, t, :], 0.0)

            psum_of = {}
            seen = {}
            for (w, off, n, chunks) in l1_pieces:
                g = g_p.tile([128, GSZ // 128, DP], f32, tag="g1")
                nc.gpsimd.dma_gather(
                    out_ap=g[:, :n // 128, :],
                    in_ap=ego[w * WIN:min((w + 1) * WIN, N_NODES), :],
                    idxs_ap=idx_sb[:, off // 16:(off + n) // 16],
                    num_idxs=n, num_idxs_reg=n, elem_size=DP,
                    queue_num=next_q(),
                )
                for (t, cloc, cglob) in chunks:
                    if t not in psum_of:
                        psum_of[t] = ps_p.tile([128, EMB], f32, tag="ps1", name="ps1t")
                        seen[t] = 0
                    s = s_p.tile([128, 128], f32, tag="s1")
                    nc.vector.tensor_scalar(
                        s[:], iota_sb[:],
                        rl_sb[:, cglob:cglob + 1],
                        val_sb[:, cglob:cglob + 1],
                        OP.is_equal, OP.mult)
                    seen[t] += 1
                    nc.tensor.matmul(
                        psum_of[t][:], s[:], g[:, cloc, :EMB],
                        start=(seen[t] == 1),
                        stop=(seen[t] == tile_nchunks[t]))
                    if seen[t] == tile_nchunks[t]:
                        nc.scalar.activation(e1_sb[:, t, :],
                                             psum_of[t][:], AF.Copy)
                        del psum_of[t]

            e1v = e1_hbm[:].rearrange("(p t) e -> p t e", p=128)
            nc.sync.dma_start(e1v[:, :, :EMB], e1_sb[:])

        # ---------------- L2 ----------------
        with ExitStack() as l2s:
            meta2 = l2s.enter_context(tc.tile_pool(name="l2meta", bufs=1))
            idx2_sb = meta2.tile([128, T2 // 16], i16, tag="idx2")
            nc.sync.dma_start(idx2_sb[:], l2_idx[:])
            rl2_sb = meta2.tile([128, T2 // 128], f32, tag="rl2")
            nc.sync.dma_start(rl2_sb[:], l2_rl[:])
            val2_sb = meta2.tile([128, T2 // 128], f32, tag="val2")
            nc.sync.dma_start(val2_sb[:], l2_val[:])

            g2_p = l2s.enter_context(tc.tile_pool(name="g2", bufs=16))
            s2_p = l2s.enter_context(tc.tile_pool(name="s2", bufs=4))
            ps2_p = l2s.enter_context(
                tc.tile_pool(name="ps2", bufs=4, space="PSUM"))
            part_p = l2s.enter_context(tc.tile_pool(name="part", bufs=1))

            for half in range(2):
                part = part_p.tile([128, HALF_T * 128], bf, tag="part")
                psum2 = {}
                seen2 = {}
                for (h, off, n, chunks) in l2_pieces:
                    if h != half:
                        continue
                    g = g2_p.tile([128, GSZ // 128, DP], f32, tag="g2")
                    nc.gpsimd.dma_gather(
                        out_ap=g[:, :n // 128, :],
                        in_ap=e1_hbm[:],
                        idxs_ap=idx2_sb[:, off // 16:(off + n) // 16],
                        num_idxs=n, num_idxs_reg=n, elem_size=DP,
                        queue_num=next_q(),
                    )
                    for (tau, cloc, cglob) in chunks:
                        tl = tau - half * HALF_T
                        if tau not in psum2:
                            psum2[tau] = ps2_p.tile([128, 128], f32,
                                                    tag="ps2", name="ps2t")
                            seen2[tau] = 0
                        s = s2_p.tile([128, 128], f32, tag="s2")
                        nc.vector.tensor_scalar(
                            s[:], iota_sb[:],
                            rl2_sb[:, cglob:cglob + 1],
                            val2_sb[:, cglob:cglob + 1],
                            OP.is_equal, OP.mult)
                        seen2[tau] += 1
                        nch = int(ngrp2[tau]) // 128
                        nc.tensor.matmul(
                            psum2[tau][:EMB, :], g[:, cloc, :EMB], s[:],
                            start=(seen2[tau] == 1),
                            stop=(seen2[tau] == nch))
                        if seen2[tau] == nch:
                            nc.scalar.activation(
                                part[:EMB, tl * 128:(tl + 1) * 128],
                                psum2[tau][:EMB, :], AF.Copy)
                            del psum2[tau]
                ndest = P // 2
                for dd in range(ndest):
                    nc.sync.dma_start(
                        a2a_in[ndest * half + dd],
                        part[:EMB, dd * PERCORE:(dd + 1) * PERCORE])

            nc.gpsimd.collective_compute(
                "AllToAll", mybir.AluOpType.bypass,
                replica_groups=[list(range(P))],
                ins=[a2a_in[:]],
                outs=[a2a_out[:]],
            )

        # ---------------- combine + MLP ----------------
        with ExitStack() as ms:
            acc_p = ms.enter_context(tc.tile_pool(name="acc", bufs=1))
            tmp_p = ms.enter_context(tc.tile_pool(name="tmp", bufs=2))
            mw_p = ms.enter_context(tc.tile_pool(name="mw", bufs=1))
            h_p = ms.enter_context(tc.tile_pool(name="h", bufs=1))
            ps1_p = ms.enter_context(
                tc.tile_pool(name="psm1", bufs=2, space="PSUM"))
            ps2m_p = ms.enter_context(
                tc.tile_pool(name="psm2", bufs=2, space="PSUM"))
            ps3_p = ms.enter_context(
                tc.tile_pool(name="psm3", bufs=2, space="PSUM"))

            acc = acc_p.tile([128, PERCORE], f32, tag="acc")
            egot = acc_p.tile([128, PERCORE], f32, tag="egot")
            nc.sync.dma_start(egot[:], ego_selT[:])
            tmp0 = tmp_p.tile([128, PERCORE], bf, tag="tmp")
            nc.sync.dma_start(tmp0[:EMB, :], a2a_out[0])
            nc.vector.tensor_tensor(acc[:EMB, :], egot[:EMB, :],
                                    tmp0[:EMB, :], op=OP.add)
            for i in range(1, P):
                tmp = tmp_p.tile([128, PERCORE], bf, tag="tmp")
                nc.sync.dma_start(tmp[:EMB, :], a2a_out[i])
                nc.vector.tensor_tensor(acc[:EMB, :], acc[:EMB, :],
                                        tmp[:EMB, :], op=OP.add)

            w1u = mw_p.tile([EMB, 64], f32, tag="w1u")
            nc.sync.dma_start(w1u[:], w1u_in[:])
            w1i = mw_p.tile([EMB, 64], f32, tag="w1i")
            nc.sync.dma_start(w1i[:], w1i_in[:])
            w2 = mw_p.tile([64, 32], f32, tag="w2")
            nc.sync.dma_start(w2[:], w2_in[:])
            w3 = mw_p.tile([32, 1], f32, tag="w3")
            nc.sync.dma_start(w3[:], w3_in[:])
            b1 = mw_p.tile([64, 1], f32, tag="b1")
            nc.sync.dma_start(b1[:], b1_in[:])
            b2 = mw_p.tile([32, 1], f32, tag="b2")
            nc.sync.dma_start(b2[:], b2_in[:])
            b3 = mw_p.tile([1, 1], f32, tag="b3")
            nc.sync.dma_start(b3[:], b3_in[:])

            NB = PERCORE // 2
            MP = min(512, NB)
            h1 = h_p.tile([64, NB], f32, tag="h1")
            h2 = h_p.tile([32, NB], f32, tag="h2")
            h3 = h_p.tile([1, NB], f32, tag="h3")
            for npi in range(NB // MP):
                sl = slice(npi * MP, (npi + 1) * MP)
                ps1 = ps1_p.tile([64, MP], f32, tag="psm1")
                nc.tensor.matmul(ps1[:], w1u[:], acc[:EMB, sl],
                                 start=True, stop=False)
                nc.tensor.matmul(
                    ps1[:], w1i[:],
                    acc[:EMB, NB + npi * MP: NB + (npi + 1) * MP],
                    start=False, stop=True)
                nc.scalar.activation(h1[:, sl], ps1[:], AF.Relu,
                                     bias=b1[:])
                ps2 = ps2m_p.tile([32, MP], f32, tag="psm2")
                nc.tensor.matmul(ps2[:], w2[:], h1[:, sl],
                                 start=True, stop=True)
                nc.scalar.activation(h2[:, sl], ps2[:], AF.Identity,
                                     bias=b2[:])
                ps3 = ps3_p.tile([1, MP], f32, tag="psm3")
                nc.tensor.matmul(ps3[:], w3[:], h2[:, sl],
                                 start=True, stop=True)
                nc.scalar.activation(h3[:, sl], ps3[:], AF.Identity,
                                     bias=b3[:])
            nc.sync.dma_start(out_d[:], h3[:])

    nc.compile()
    return nc


# ======================================================================
# entry point
# ======================================================================

def kernel(**inputs):
    from concourse.bass_utils import run_bass_kernel_spmd

    sched, per_core = build_host_data(**inputs)
    nc = build_program(sched)

    if _TRACE:
        _install_ntff_hook()
    res = run_bass_kernel_spmd(nc, per_core, core_ids=list(range(P)),
                               trace=_TRACE)
    LAST_EXEC_NS[0] = res.exec_time_ns
    out = np.concatenate([res.results[m]["out"].reshape(-1)
                          for m in range(P)])
    return out.astype(np.float32)


def _install_ntff_hook():
    import types
    if "antenv.axon_hooks" not in sys.modules:
        mod = types.ModuleType("antenv.axon_hooks")
        _h = [None]
        mod.set_axon_ntff_profile_hook = lambda h: _h.__setitem__(0, h)
        mod.get_axon_ntff_profile_hook = lambda: _h[0]
        sys.modules["antenv.axon_hooks"] = mod
        import antenv
        antenv.axon_hooks = mod
    import antenv.axon_hooks as ah
    if ah.get_axon_ntff_profile_hook() is None:
        from trn_agent_boot.trn_boot import _ntff_profile_via_ctypes
        ah.set_axon_ntff_profile_hook(
            _ntff_profile_via_ctypes("/opt/axon/libaxon_pjrt.so"))
